# revision 1
# baseline (speedup 1.0000x reference)
"""Graphormer layer on 8 TRN2 NeuronCores.

Sharding: core c handles batch b = c//2 and query-row half qh = c%2 (1024 q
rows). All compute is in transposed (feature-on-partition) layout; the host
pre-transposes x and the influence slices and transposes per-core outputs
back during the gather. Host also rolls the node axis per core so each core's
own q rows sit at columns [0, 1024) — the device program is identical across
cores (attention over all keys is permutation-invariant; the influence k axis
is rolled identically).

Per core:
  ln1T = LayerNorm_T(xT)              (partition-dim LN via ones-matmul sums)
  QT/KT (f32r matmuls), V natural (bf16)
  per (qc, kc, head-pair):
    psum  = LG[kc]                    (PE identity-copy;  LG = iw1*u + ib1)
    psum += KT_h-slice.T @ QT_h-slice (f32r, contract d=32, row-packed heads)
    e = exp(psum / sqrt(D))           (ACT -> SBUF bf16; includes influence add)
    f = e * G2[kc]                    (DVE bf16;  G2 = iw2*u + ib2)
    wv += V_kc-slice.T @ f            (bf16, col-packed per head)
    Z  += ones.T @ e                  (bf16, col-packed per head)
  attn = (wv / Z) @ Wo + bo ; h = attn + xT_own
  out = W2.T-proj(relu(W1.T-proj(LN_T(h)) + b1)) + b2 + h
"""

import math

import numpy as np

import concourse.bass as bass
import concourse.bacc as bacc
import concourse.mybir as mybir
import concourse.tile as tile
from concourse.bass_utils import run_bass_kernel_spmd

B, N, E, H, D = 4, 2048, 256, 8, 32
NQ = N // 2          # q rows per core
QC = 512             # q window
NKC = N // 128       # 16 k-chunks
EC = E // 128        # 2 feature chunks

f32 = mybir.dt.float32
f32r = mybir.dt.float32r
bf16 = mybir.dt.bfloat16
FT = mybir.ActivationFunctionType
ALU = mybir.AluOpType

# vecs_sb column index: vec v, chunk c -> 2*v + c
V_G1, V_BETA1, V_G2, V_BETA2, V_BO, V_B1, V_B2 = range(7)


def r32(ap):
    return ap.bitcast(f32r)


def layer_norm_T(nc, pp, ps, x_chunks, win, wn, g_col, beta_col, vecs, ones,
                 eps_ap, out_chunks):
    """LayerNorm over the partition dim (E = 2 chunks) in T layout.

    x_chunks: 2 SBUF APs; normalizes cols [win:win+wn] -> out_chunks (f32).
    """
    sq = ps.tile([128, 2 * wn], f32, name="lnsq", tag="lnsq")
    p_s = pp.tile([128, wn], f32, name="lnps", tag="lnps")
    p_sq = pp.tile([128, wn], f32, name="lnpsq", tag="lnpsq")
    for c in range(EC):
        xs = x_chunks[c][:, win:win + wn]
        nc.vector.tensor_mul(sq[:, c * wn:(c + 1) * wn], xs, xs)
        nc.tensor.matmul(p_s[:, :], ones[:, :], xs,
                         start=(c == 0), stop=(c == EC - 1))
    for c in range(EC):
        nc.tensor.matmul(p_sq[:, :], ones[:, :],
                         sq[:, c * wn:(c + 1) * wn],
                         start=(c == 0), stop=(c == EC - 1))
    mu = ps.tile([128, wn], f32, name="lnmu", tag="lnmu")
    msq = ps.tile([128, wn], f32, name="lnmsq", tag="lnmsq")
    nc.vector.tensor_scalar_mul(mu[:, :], p_s[:, :], 1.0 / E)
    nc.vector.tensor_scalar_mul(msq[:, :], p_sq[:, :], 1.0 / E)
    mu2 = ps.tile([128, wn], f32, name="lnmu2", tag="lnmu2")
    nc.vector.tensor_mul(mu2[:, :], mu[:, :], mu[:, :])
    var = ps.tile([128, wn], f32, name="lnvar", tag="lnvar")
    nc.vector.tensor_sub(var[:, :], msq[:, :], mu2[:, :])
    sd = ps.tile([128, wn], f32, name="lnsd", tag="lnsd")
    nc.scalar.activation(sd[:, :], var[:, :], FT.Sqrt, bias=eps_ap)
    rstd = ps.tile([128, wn], f32, name="lnrstd", tag="lnrstd")
    nc.vector.reciprocal_approx_fast(rstd[:, :], sd[:, :])
    for c in range(EC):
        xs = x_chunks[c][:, win:win + wn]
        xm = ps.tile([128, wn], f32, name="lnxm", tag="lnxm")
        nc.vector.tensor_sub(xm[:, :], xs, mu[:, :])
        xm2 = ps.tile([128, wn], f32, name="lnxm2", tag="lnxm2")
        nc.vector.tensor_mul(xm2[:, :], xm[:, :], rstd[:, :])
        nc.vector.tensor_scalar(
            out_chunks[c][:, win:win + wn], xm2[:, :],
            vecs[:, 2 * g_col + c:2 * g_col + c + 1],
            vecs[:, 2 * beta_col + c:2 * beta_col + c + 1],
            ALU.mult, ALU.add)


def build_body(nc, tc, xT_d, inflT_d, w_d, vecs_d, scal_d, ident_d, outT_d):
    persist_pools = []

    def ppool(name):
        p = tc.tile_pool(name=name, bufs=1)
        persist_pools.append(p)
        return p.__enter__()

    persist = ppool("persist")

    # ---- persistent SBUF ----
    qt = [persist.tile([128, NQ], bf16, name=f"qt{c}", tag=f"qt{c}") for c in range(EC)]
    kt = [persist.tile([128, N], bf16, name=f"kt{c}", tag=f"kt{c}") for c in range(EC)]
    xtq = [persist.tile([128, NQ], f32, name=f"xtq{c}", tag=f"xtq{c}") for c in range(EC)]
    v_sb = [persist.tile([128, E], bf16, name=f"v{k}", tag=f"v{k}") for k in range(NKC)]
    ga_sb = [persist.tile([128, NQ], bf16, name=f"ga_{k}", tag=f"ga_{k}") for k in range(NKC)]
    gb_sb = [persist.tile([128, NQ], bf16, name=f"gb_{k}", tag=f"gb_{k}") for k in range(NKC)]
    id_bf = persist.tile([128, 128], bf16, name="id_bf", tag="id_bf")
    w_sb = {n: persist.tile([128, 2 * E], f32, name=f"w_{n}", tag=f"w_{n}") for n in w_d}
    w_bf = {n: persist.tile([128, 2 * E], bf16, name=f"wbf_{n}", tag=f"wbf_{n}")
            for n in w_d}
    vecs = persist.tile([128, 14], f32, name="vecs", tag="vecs")
    scal = persist.tile([128, 4], f32, name="scal", tag="scal")
    ones = persist.tile([128, 128], f32, name="ones", tag="ones")
    ones_bf = persist.tile([128, 32], bf16, name="ones_bf", tag="ones_bf")
    h_sb = [[persist.tile([128, QC], f32, name=f"h{q}{c}", tag=f"h{q}{c}") for c in range(EC)]
            for q in range(2)]

    # ---- small loads ----
    for n in w_d:
        for c in range(EC):
            nc.sync.dma_start(w_sb[n][:, E * c:E * (c + 1)],
                              w_d[n][128 * c:128 * (c + 1), :])
    nc.sync.dma_start(vecs[:, :], vecs_d[:, :])
    nc.sync.dma_start(scal[:, :], scal_d[:, :])
    idt = persist.tile([128, 128], f32, name="id_f32", tag="id_f32")
    nc.sync.dma_start(idt[:, :], ident_d[:, :])
    nc.vector.tensor_copy(id_bf[:, :], idt[:, :])
    eps_t = persist.tile([128, 1], f32, name="eps_t", tag="eps_t")
    nc.vector.memset(eps_t[:, :], 1e-5)
    nc.vector.memset(ones[:, :], 1.0)
    nc.vector.memset(ones_bf[:, :], 1.0)
    for n in w_d:
        nc.vector.tensor_copy(w_bf[n][:, :], w_sb[n][:, :])

    # ---- stage B/C: LN1 + projections (xt/ln1 are stage-local) ----
    with tc.tile_pool(name="xt_pool", bufs=1) as xp, \
         tc.tile_pool(name="ln_psum", bufs=2, space="PSUM") as ln_pp, \
         tc.tile_pool(name="ln_sbuf", bufs=2) as ln_ps, \
         tc.tile_pool(name="proj_psum", bufs=2, space="PSUM") as proj_psum:
        xt = [xp.tile([128, N], f32, name=f"xt{c}", tag=f"xt{c}") for c in range(EC)]
        ln1 = [xp.tile([128, N], bf16, name=f"ln1{c}", tag=f"ln1{c}") for c in range(EC)]
        for c in range(EC):
            nc.sync.dma_start(xt[c][:, :], xT_d[128 * c:128 * (c + 1), :])
            nc.vector.tensor_copy(xtq[c][:, :], xt[c][:, :NQ])
        for w in range(N // 512):
            layer_norm_T(nc, ln_pp, ln_ps, xt, 512 * w, 512, V_G1, V_BETA1,
                         vecs, ones, eps_t[:, :], ln1)
        for fc in range(EC):
            for qw in range(NQ // 512):
                pq = proj_psum.tile([128, 512], f32, name="proj", tag="proj")
                for ec in range(EC):
                    nc.tensor.matmul(
                        pq[:, :],
                        w_bf["Wq"][:, E * ec + 128 * fc:E * ec + 128 * (fc + 1)],
                        ln1[ec][:, 512 * qw:512 * (qw + 1)],
                        start=(ec == 0), stop=(ec == EC - 1))
                nc.vector.tensor_copy(qt[fc][:, 512 * qw:512 * (qw + 1)], pq[:, :])
        for fc in range(EC):
            for kw in range(N // 512):
                pk = proj_psum.tile([128, 512], f32, name="proj", tag="proj")
                for ec in range(EC):
                    nc.tensor.matmul(
                        pk[:, :],
                        w_bf["Wk"][:, E * ec + 128 * fc:E * ec + 128 * (fc + 1)],
                        ln1[ec][:, 512 * kw:512 * (kw + 1)],
                        start=(ec == 0), stop=(ec == EC - 1))
                nc.vector.tensor_copy(kt[fc][:, 512 * kw:512 * (kw + 1)], pk[:, :])
        for k in range(NKC):
            pv = proj_psum.tile([128, E], f32, name="projv", tag="projv")
            for ec in range(EC):
                nc.tensor.matmul(
                    pv[:, :],
                    ln1[ec][:, 128 * k:128 * (k + 1)],
                    w_bf["Wv"][:, E * ec:E * (ec + 1)],
                    start=(ec == 0), stop=(ec == EC - 1))
            nc.vector.tensor_copy(v_sb[k][:, :], pv[:, :])

    # ---- stage D (hybrid): even kc -> LG,G2 ; odd kc -> EG,G3 (bf16) ----
    with tc.tile_pool(name="gprep", bufs=3) as gp:
        for k in range(NKC):
            u = gp.tile([128, NQ], f32, name="u", tag="u")
            nc.sync.dma_start(u[:, :], inflT_d[128 * k:128 * (k + 1), :])
            if k % 2 == 0:
                nc.vector.tensor_scalar(ga_sb[k][:, :], u[:, :], scal[:, 0:1],
                                        scal[:, 1:2], ALU.mult, ALU.add)
                nc.vector.tensor_scalar(gb_sb[k][:, :], u[:, :], scal[:, 2:3],
                                        scal[:, 3:4], ALU.mult, ALU.add)
            else:
                nc.scalar.activation(ga_sb[k][:, :], u[:, :], FT.Exp,
                                     scale=scal[:, 0:1], bias=scal[:, 1:2])
                g2t = gp.tile([128, NQ], bf16, name="g2t", tag="g2t")
                nc.vector.tensor_scalar(g2t[:, :], u[:, :], scal[:, 2:3],
                                        scal[:, 3:4], ALU.mult, ALU.add)
                nc.vector.tensor_mul(gb_sb[k][:, :], ga_sb[k][:, :], g2t[:, :])

    # ---- stage E: attention ----
    inv_sqrt_d = 1.0 / math.sqrt(D)
    with tc.tile_pool(name="score_psum", bufs=2, space="PSUM") as sp, \
         tc.tile_pool(name="acc_psum", bufs=1, space="PSUM") as ap_, \
         tc.tile_pool(name="ef_sbuf", bufs=6) as efp, \
         tc.tile_pool(name="att_sbuf", bufs=2) as asb:
        for qc in range(2):
            q0 = QC * qc
            wv_ps = [ap_.tile([128, QC], f32, name=f"wv{s}", tag=f"wv{s}") for s in range(2)]
            z_ps = [ap_.tile([128, QC], f32, name=f"z{s}", tag=f"z{s}") for s in range(2)]
            for kc in range(NKC):
                for half in range(2):  # head sets {0-3}, {4-7}
                    sts = []
                    for hg in (2 * half, 2 * half + 1):
                        st = sp.tile([128, 2 * QC], f32, name="score", tag="score")
                        sts.append((st, hg))
                    even = (kc % 2 == 0)
                    if even:
                        # LG preloads (full-array identity copies, keeps PE hot)
                        for st, hg in sts:
                            for j in range(2):
                                nc.tensor.matmul(
                                    st[:, QC * j:QC * (j + 1)],
                                    id_bf[:, :],
                                    ga_sb[kc][:, q0:q0 + QC],
                                    start=True, stop=False)
                    # 4 q.k matmuls back-to-back on distinct row-groups -> pack
                    for st, hg in sts:
                        for j in range(2):
                            h = 2 * hg + j
                            c, hh = h // 4, 32 * (h % 4)
                            nc.tensor.matmul(
                                st[:, QC * j:QC * (j + 1)],
                                kt[c][hh:hh + 32, 128 * kc:128 * (kc + 1)],
                                qt[c][hh:hh + 32, q0:q0 + QC],
                                start=not even, stop=True,
                                skip_group_check=True, tile_position=(hh, 0))
                    gab = ga_sb[kc][:, q0:q0 + QC].rearrange(
                        "p (o q) -> p o q", o=1).broadcast_to([128, 2, QC])
                    gbb = gb_sb[kc][:, q0:q0 + QC].rearrange(
                        "p (o q) -> p o q", o=1).broadcast_to([128, 2, QC])
                    for st, hg in sts:
                        e = efp.tile([128, 2 * QC], bf16, name="e", tag="e")
                        nc.scalar.activation(e[:, :], st[:, :], FT.Exp)
                        er = e[:, :].rearrange("p (o q) -> p o q", o=2)
                        if even:
                            zsrc = e    # e already includes the influence add
                        else:
                            zsrc = efp.tile([128, 2 * QC], bf16, name="t", tag="t")
                            nc.vector.tensor_tensor(
                                zsrc[:, :].rearrange("p (o q) -> p o q", o=2),
                                er, gab, ALU.mult)
                        for j in range(2):
                            h = 2 * hg + j
                            s_, hh = h // 4, 32 * (h % 4)
                            nc.tensor.matmul(
                                z_ps[s_][hh:hh + 32, :],
                                ones_bf[:, :],
                                zsrc[:, QC * j:QC * (j + 1)],
                                start=(kc == 0), stop=(kc == NKC - 1),
                                skip_group_check=True, tile_position=(0, hh))
                        f = efp.tile([128, 2 * QC], bf16, name="f", tag="f")
                        nc.vector.tensor_tensor(
                            f[:, :].rearrange("p (o q) -> p o q", o=2),
                            er, gbb, ALU.mult)
                        for j in range(2):
                            h = 2 * hg + j
                            s_, hh = h // 4, 32 * (h % 4)
                            nc.tensor.matmul(
                                wv_ps[s_][hh:hh + 32, :],
                                v_sb[kc][:, 32 * h:32 * h + 32],
                                f[:, QC * j:QC * (j + 1)],
                                start=(kc == 0), stop=(kc == NKC - 1),
                                skip_group_check=True, tile_position=(0, hh))
            # normalize + Wo projection + bias + residual -> h
            on = []
            for s in range(2):
                zr = asb.tile([128, QC], f32, name=f"zr{s}", tag=f"zr{s}")
                nc.vector.reciprocal_approx_fast(zr[:, :], z_ps[s][:, :])
                o = asb.tile([128, QC], bf16, name=f"on{s}", tag=f"on{s}")
                nc.vector.tensor_mul(o[:, :], wv_ps[s][:, :], zr[:, :])
                on.append(o)
            for fc in range(EC):
                po = sp.tile([128, QC], f32, name="score", tag="score")
                for ec in range(EC):
                    nc.tensor.matmul(
                        po[:, :],
                        w_bf["Wo"][:, E * ec + 128 * fc:E * ec + 128 * (fc + 1)],
                        on[ec][:, :],
                        start=(ec == 0), stop=(ec == EC - 1))
                ta = asb.tile([128, QC], f32, name="tattn", tag="tattn")
                nc.vector.tensor_scalar_add(ta[:, :], po[:, :],
                                            vecs[:, 2 * V_BO + fc:2 * V_BO + fc + 1])
                nc.vector.tensor_add(h_sb[qc][fc][:, :], ta[:, :],
                                     xtq[fc][:, q0:q0 + QC])

    # ---- stage F: LN2 + FFN + residual + store ----
    with tc.tile_pool(name="ln_psum2", bufs=2, space="PSUM") as ln_pp2, \
         tc.tile_pool(name="ln_sbuf2", bufs=2) as ln_ps2, \
         tc.tile_pool(name="ffn_psum", bufs=2, space="PSUM") as fp, \
         tc.tile_pool(name="ffn_sbuf", bufs=2) as fs:
        for qc in range(2):
            ln2 = [fs.tile([128, QC], bf16, name=f"ln2{c}", tag=f"ln2{c}") for c in range(EC)]
            layer_norm_T(nc, ln_pp2, ln_ps2, h_sb[qc], 0, QC, V_G2, V_BETA2,
                         vecs, ones, eps_t[:, :], ln2)
            z1 = [fs.tile([128, QC], bf16, name=f"z1{c}", tag=f"z1{c}") for c in range(EC)]
            for fc in range(EC):
                p1 = fp.tile([128, QC], f32, name="ffn", tag="ffn")
                for ec in range(EC):
                    nc.tensor.matmul(
                        p1[:, :],
                        w_bf["W1"][:, E * ec + 128 * fc:E * ec + 128 * (fc + 1)],
                        ln2[ec][:, :],
                        start=(ec == 0), stop=(ec == EC - 1))
                nc.vector.tensor_scalar(z1[fc][:, :], p1[:, :],
                                        vecs[:, 2 * V_B1 + fc:2 * V_B1 + fc + 1],
                                        0.0, ALU.add, ALU.max)
            for fc in range(EC):
                p2 = fp.tile([128, QC], f32, name="ffn", tag="ffn")
                for ec in range(EC):
                    nc.tensor.matmul(
                        p2[:, :],
                        w_bf["W2"][:, E * ec + 128 * fc:E * ec + 128 * (fc + 1)],
                        z1[ec][:, :],
                        start=(ec == 0), stop=(ec == EC - 1))
                t2 = fs.tile([128, QC], f32, name="t2", tag="t2")
                nc.vector.tensor_scalar_add(t2[:, :], p2[:, :],
                                            vecs[:, 2 * V_B2 + fc:2 * V_B2 + fc + 1])
                of = fs.tile([128, QC], f32, name="of", tag="of")
                nc.vector.tensor_add(of[:, :], t2[:, :], h_sb[qc][fc][:, :])
                nc.sync.dma_start(
                    outT_d[128 * fc:128 * (fc + 1), QC * qc:QC * (qc + 1)],
                    of[:, :])

    for p in reversed(persist_pools):
        p.__exit__(None, None, None)


def build_nc():
    nc = bacc.Bacc(
        "TRN2",
        target_bir_lowering=False,
        debug=False,
        enable_asserts=False,
        num_devices=8,
    )
    xT_d = nc.dram_tensor("xT", [E, N], f32, kind="ExternalInput").ap()
    inflT_d = nc.dram_tensor("inflT", [N, NQ], f32, kind="ExternalInput").ap()
    w_d = {
        name: nc.dram_tensor(name, [E, E], f32, kind="ExternalInput").ap()
        for name in ("Wq", "Wk", "Wv", "Wo", "W1", "W2")
    }
    vecs_d = nc.dram_tensor("vecs", [128, 14], f32, kind="ExternalInput").ap()
    scal_d = nc.dram_tensor("scal", [128, 4], f32, kind="ExternalInput").ap()
    ident_d = nc.dram_tensor("ident", [128, 128], f32, kind="ExternalInput").ap()
    outT_d = nc.dram_tensor("outT", [E, NQ], f32, kind="ExternalOutput").ap()

    with tile.TileContext(nc) as tc:
        build_body(nc, tc, xT_d, inflT_d, w_d, vecs_d, scal_d, ident_d, outT_d)
    nc.compile()
    return nc


def host_shard(inputs):
    """Build the 8 per-core input maps (see module docstring for the roll)."""
    x = np.asarray(inputs["x"], np.float32)
    infl = np.asarray(inputs["influence_matrix"], np.float32)
    vec_list = ["g1", "beta1", "g2", "beta2", "bo", "b1", "b2"]
    vecs_np = np.empty((128, 14), np.float32)
    for vi, nm in enumerate(vec_list):
        v = np.asarray(inputs[nm], np.float32).reshape(E)
        vecs_np[:, 2 * vi] = v[:128]
        vecs_np[:, 2 * vi + 1] = v[128:]
    scal_np = np.tile(
        np.array([inputs["iw1"], inputs["ib1"], inputs["iw2"], inputs["ib2"]],
                 np.float32).reshape(1, 4), (128, 1))
    # Fold the 1/sqrt(D) score scale into Q host-side? No: fold into Wq here.
    ws = {n: np.ascontiguousarray(np.asarray(inputs[n], np.float32))
          for n in ("Wq", "Wk", "Wv", "Wo", "W1", "W2")}
    ws["Wq"] = ws["Wq"] / math.sqrt(D)

    in_maps = []
    for core in range(8):
        b, qh = core // 2, core % 2
        qoff = qh * NQ
        xb = np.roll(x[b], -qoff, axis=0)          # [N, E], own rows first
        xT = np.ascontiguousarray(xb.T)            # [E, N]
        inf_slice = np.roll(infl[b][qoff:qoff + NQ, :], -qoff, axis=1)
        inflT = np.ascontiguousarray(inf_slice.T)  # [N(k), NQ]
        m = {"xT": xT, "inflT": inflT, "vecs": vecs_np, "scal": scal_np,
             "ident": np.eye(128, dtype=np.float32)}
        m.update(ws)
        in_maps.append(m)
    return in_maps


_NC_CACHE = []


def kernel(**inputs):
    if not _NC_CACHE:
        _NC_CACHE.append(build_nc())
    nc = _NC_CACHE[0]
    in_maps = host_shard(inputs)
    res = run_bass_kernel_spmd(nc, in_maps, core_ids=list(range(8)))
    out = np.empty((B, N, E), np.float32)
    for core in range(8):
        b, qh = core // 2, core % 2
        out[b, qh * NQ:(qh + 1) * NQ, :] = np.asarray(
            res.results[core]["outT"], np.float32).T
    return out



# revision 6
# speedup vs baseline: 1.1000x; 1.1000x over previous
"""Graphormer layer on 8 TRN2 NeuronCores — v2 (ACT-bound redesign).

Sharding: core c handles batch b = c//2 and query-row half qh = c%2 (1024 q
rows). Transposed (feature-on-partition) layout throughout; host pre-rolls
the node axis per core so each core's own q rows sit first, and ships:
  - ln1T  : LayerNorm1(x) pre-computed, transposed, bf16
  - xqT   : x^T own-query columns (residual), f32
  - lgT   : (iw1*infl + ib1)^T per-core slice, bf16  (score bias)
  - g2T   : (iw2*infl + ib2)^T per-core slice, bf16  (post-softmax gate)
  - weights bf16 (Wq pre-scaled by 1/sqrt(D))

Device per core:
  B:  QT/KT/V projections from ln1T (bf16 matmuls)
  E:  per (qc, kc): S[128,2048] psum = LG preload (identity matmul) +
      packed per-head QK matmuls; e = exp(S) (ACT, the bottleneck ~118us);
      f = e*G2 (DVE 2x); Z += ones^T e, wv += V^T f (col-packed matmuls,
      emitted with one-iteration lag to keep PE FIFO stall-free)
  F:  attn = (wv/Z) @ Wo + bo + x ; LN2 + FFN + residual, store.
"""

import math

import numpy as np
import ml_dtypes

import concourse.bass as bass
import concourse.bacc as bacc
import concourse.mybir as mybir
import concourse.tile as tile
from concourse.bass_utils import run_bass_kernel_spmd

B, N, E, H, D = 4, 2048, 256, 8, 32
NQ = N // 2          # q rows per core
QC = 512             # q window
NKC = N // 128       # 16 k-chunks
EC = E // 128        # 2 feature chunks

f32 = mybir.dt.float32
bf16 = mybir.dt.bfloat16
FT = mybir.ActivationFunctionType
ALU = mybir.AluOpType

# vecs_sb column index: vec v, chunk c -> 2*v + c
V_G2, V_BETA2, V_BO, V_B1, V_B2 = range(5)
W_NAMES = ("Wq", "Wk", "Wv", "Wo", "W1", "W2")


def build_body(nc, tc, ln1T_d, xqT_d, lgT_d, g2T_d, w_d, vecs_d, ident_d,
               outT_d):
    persist_pools = []

    def ppool(name, space="SBUF"):
        p = tc.tile_pool(name=name, bufs=1, space=space)
        persist_pools.append(p)
        return p.__enter__()

    persist = ppool("persist")

    # ---- persistent SBUF ----
    qt = [persist.tile([128, NQ], bf16, name=f"qt{c}", tag=f"qt{c}") for c in range(EC)]
    kt = [persist.tile([128, N], bf16, name=f"kt{c}", tag=f"kt{c}") for c in range(EC)]
    xtq = [persist.tile([128, NQ], f32, name=f"xtq{c}", tag=f"xtq{c}") for c in range(EC)]
    v_sb = [persist.tile([128, E], bf16, name=f"v{k}", tag=f"v{k}") for k in range(NKC)]
    id_bf = persist.tile([128, 128], bf16, name="id_bf", tag="id_bf")
    w_bf = {n: persist.tile([128, 2 * E], bf16, name=f"w_{n}", tag=f"w_{n}")
            for n in W_NAMES}
    vecs = persist.tile([128, 10], f32, name="vecs", tag="vecs")
    ones = persist.tile([128, 128], f32, name="ones", tag="ones")
    ones_bf = persist.tile([128, 32], bf16, name="ones_bf", tag="ones_bf")
    ln1 = [persist.tile([128, N], bf16, name=f"ln1{c}", tag=f"ln1{c}") for c in range(EC)]
    h_sb = [persist.tile([128, NQ], f32, name=f"h{c}", tag=f"h{c}") for c in range(EC)]

    # ---- small loads ----
    for n in W_NAMES:
        for c in range(EC):
            nc.sync.dma_start(w_bf[n][:, E * c:E * (c + 1)],
                              w_d[n][128 * c:128 * (c + 1), :])
    nc.sync.dma_start(vecs[:, :], vecs_d[:, :])
    nc.sync.dma_start(id_bf[:, :], ident_d[:, :])
    for c in range(EC):
        nc.sync.dma_start(ln1[c][:, :], ln1T_d[128 * c:128 * (c + 1), :])
        nc.sync.dma_start(xtq[c][:, :], xqT_d[128 * c:128 * (c + 1), :])
    nc.vector.memset(ones[:, :], 1.0)
    nc.vector.memset(ones_bf[:, :], 1.0)
    eps_t = persist.tile([128, 1], f32, name="eps_t", tag="eps_t")
    nc.vector.memset(eps_t[:, :], 1e-5)
    zero_t = persist.tile([128, 1], f32, name="zero_t", tag="zero_t")
    nc.vector.memset(zero_t[:, :], 0.0)

    # ---- stage B: Q/K/V projections ----
    with tc.tile_pool(name="proj_psum", bufs=2, space="PSUM") as proj_psum:
        for fc in range(EC):
            for qw in range(NQ // 512):
                pq = proj_psum.tile([128, 512], f32, name="proj", tag="proj")
                for ec in range(EC):
                    nc.tensor.matmul(
                        pq[:, :],
                        w_bf["Wq"][:, E * ec + 128 * fc:E * ec + 128 * (fc + 1)],
                        ln1[ec][:, 512 * qw:512 * (qw + 1)],
                        start=(ec == 0), stop=(ec == EC - 1))
                nc.vector.tensor_copy(qt[fc][:, 512 * qw:512 * (qw + 1)], pq[:, :])
        for fc in range(EC):
            for kw in range(N // 512):
                pk = proj_psum.tile([128, 512], f32, name="proj", tag="proj")
                for ec in range(EC):
                    nc.tensor.matmul(
                        pk[:, :],
                        w_bf["Wk"][:, E * ec + 128 * fc:E * ec + 128 * (fc + 1)],
                        ln1[ec][:, 512 * kw:512 * (kw + 1)],
                        start=(ec == 0), stop=(ec == EC - 1))
                nc.vector.tensor_copy(kt[fc][:, 512 * kw:512 * (kw + 1)], pk[:, :])
        for k in range(NKC):
            pv = proj_psum.tile([128, E], f32, name="projv", tag="projv")
            for ec in range(EC):
                nc.tensor.matmul(
                    pv[:, :],
                    ln1[ec][:, 128 * k:128 * (k + 1)],
                    w_bf["Wv"][:, E * ec:E * (ec + 1)],
                    start=(ec == 0), stop=(ec == EC - 1))
            nc.vector.tensor_copy(v_sb[k][:, :], pv[:, :])

    # ---- stage E: attention ----
    # PSUM: S (4 banks) + wv (2) + z (2). S is single-buffered but consumed
    # in two 2-bank halves so next-iteration preloads only wait on the
    # matching half's exp. Z/wv matmuls are emitted one iteration late so
    # the PE FIFO never stalls behind ACT/DVE.
    with tc.tile_pool(name="score_psum", bufs=1, space="PSUM") as sp, \
         tc.tile_pool(name="acc_psum", bufs=1, space="PSUM") as ap_, \
         tc.tile_pool(name="ef_sbuf", bufs=3) as efp, \
         tc.tile_pool(name="g_sbuf", bufs=4) as gp, \
         tc.tile_pool(name="att_sbuf", bufs=2) as asb:
        S = sp.tile([128, 4 * QC], f32, name="S", tag="S")
        for qc in range(2):
            q0 = QC * qc
            wv_ps = [ap_.tile([128, QC], f32, name=f"wv{s}", tag=f"wv{s}")
                     for s in range(2)]
            z_ps = [ap_.tile([128, QC], f32, name=f"z{s}", tag=f"z{s}")
                    for s in range(2)]
            pending = []
            for kc in range(NKC):
                lg_t = gp.tile([128, QC], bf16, name="lg", tag="lg")
                g2_t = gp.tile([128, QC], bf16, name="g2", tag="g2")
                nc.sync.dma_start(lg_t[:, :], lgT_d[128 * kc:128 * (kc + 1),
                                                   q0:q0 + QC])
                nc.sync.dma_start(g2_t[:, :], g2T_d[128 * kc:128 * (kc + 1),
                                                    q0:q0 + QC])
                for hg in range(2):
                    c = hg  # head group hg covers heads 4hg..4hg+3 = chunk hg
                    # PE: half A (heads 0,1 of group) then half B (2,3);
                    # exp of half A overlaps PE work on half B.
                    for half in range(2):
                        for j in (2 * half, 2 * half + 1):
                            nc.tensor.matmul(
                                S[:, QC * j:QC * (j + 1)],
                                id_bf[:, :], lg_t[:, :],
                                start=True, stop=False)
                        for j in (2 * half, 2 * half + 1):
                            hh = 32 * j
                            nc.tensor.matmul(
                                S[:, QC * j:QC * (j + 1)],
                                kt[c][hh:hh + 32, 128 * kc:128 * (kc + 1)],
                                qt[c][hh:hh + 32, q0:q0 + QC],
                                start=False, stop=True,
                                skip_group_check=True, tile_position=(hh, 0))
                    # flush previous iteration's Z/wv matmuls (deps all met)
                    for fn in pending:
                        fn()
                    pending = []
                    e = efp.tile([128, 4 * QC], bf16, name="e", tag="e")
                    f = efp.tile([128, 4 * QC], bf16, name="f", tag="f")
                    g2b = g2_t[:, :].rearrange("p (o q) -> p o q", o=1)\
                        .broadcast_to([128, 2, QC])
                    for half in range(2):
                        sl = slice(2 * QC * half, 2 * QC * (half + 1))
                        nc.scalar.activation(e[:, sl], S[:, sl], FT.Exp,
                                             bias=zero_t[:, :])
                        nc.vector.tensor_tensor(
                            f[:, sl].rearrange("p (o q) -> p o q", o=2),
                            e[:, sl].rearrange("p (o q) -> p o q", o=2),
                            g2b, ALU.mult)

                    def mk(kc=kc, hg=hg, e=e, f=f):
                        def emit():
                            for j in range(4):
                                nc.tensor.matmul(
                                    z_ps[hg][32 * j:32 * j + 32, :],
                                    ones_bf[:, :],
                                    e[:, QC * j:QC * (j + 1)],
                                    start=(kc == 0), stop=(kc == NKC - 1),
                                    skip_group_check=True,
                                    tile_position=(0, 32 * j))
                            for j in range(4):
                                h = 4 * hg + j
                                nc.tensor.matmul(
                                    wv_ps[hg][32 * j:32 * j + 32, :],
                                    v_sb[kc][:, 32 * h:32 * h + 32],
                                    f[:, QC * j:QC * (j + 1)],
                                    start=(kc == 0), stop=(kc == NKC - 1),
                                    skip_group_check=True,
                                    tile_position=(0, 32 * j))
                        return emit
                    pending.append(mk())
            for fn in pending:
                fn()
            # normalize + Wo projection + bias + residual -> h
            on = []
            for s in range(2):
                zr = asb.tile([128, QC], f32, name=f"zr{s}", tag=f"zr{s}")
                nc.vector.reciprocal_approx_fast(zr[:, :], z_ps[s][:, :])
                o = asb.tile([128, QC], bf16, name=f"on{s}", tag=f"on{s}")
                nc.vector.tensor_mul(o[:, :], wv_ps[s][:, :], zr[:, :])
                on.append(o)
            for fc in range(EC):
                po = sp.tile([128, QC], f32, name="S", tag="S")
                for ec in range(EC):
                    nc.tensor.matmul(
                        po[:, :],
                        w_bf["Wo"][:, E * ec + 128 * fc:E * ec + 128 * (fc + 1)],
                        on[ec][:, :],
                        start=(ec == 0), stop=(ec == EC - 1))
                nc.vector.scalar_tensor_tensor(
                    h_sb[fc][:, q0:q0 + QC], po[:, :],
                    vecs[:, 2 * V_BO + fc:2 * V_BO + fc + 1],
                    xtq[fc][:, q0:q0 + QC], ALU.add, ALU.add)

    # ---- stage F: LN2 + FFN + residual + store ----
    with tc.tile_pool(name="ln_psum2", bufs=2, space="PSUM") as ln_pp2, \
         tc.tile_pool(name="ffn_psum", bufs=2, space="PSUM") as fp, \
         tc.tile_pool(name="ffn_sbuf", bufs=2) as fs:
        # LN2 over partition dim (E = 2 chunks), full NQ width
        sq = [fs.tile([128, NQ], f32, name=f"sq{c}", tag=f"sq{c}") for c in range(EC)]
        for c in range(EC):
            nc.vector.tensor_mul(sq[c][:, :], h_sb[c][:, :], h_sb[c][:, :])
        p_s = [ln_pp2.tile([128, 512], f32, name="lnps", tag="lnps")
               for _ in range(2)]
        p_sq = [ln_pp2.tile([128, 512], f32, name="lnpsq", tag="lnpsq")
                for _ in range(2)]
        for w in range(2):
            for c in range(EC):
                nc.tensor.matmul(p_s[w][:, :], ones[:, :],
                                 h_sb[c][:, 512 * w:512 * (w + 1)],
                                 start=(c == 0), stop=(c == EC - 1))
            for c in range(EC):
                nc.tensor.matmul(p_sq[w][:, :], ones[:, :],
                                 sq[c][:, 512 * w:512 * (w + 1)],
                                 start=(c == 0), stop=(c == EC - 1))
        mu = fs.tile([128, NQ], f32, name="mu", tag="mu")
        msq = fs.tile([128, NQ], f32, name="msq", tag="msq")
        for w in range(2):
            nc.vector.tensor_scalar_mul(mu[:, 512 * w:512 * (w + 1)],
                                        p_s[w][:, :], 1.0 / E)
            nc.vector.tensor_scalar_mul(msq[:, 512 * w:512 * (w + 1)],
                                        p_sq[w][:, :], 1.0 / E)
        mu2 = fs.tile([128, NQ], f32, name="mu2", tag="mu2")
        nc.vector.tensor_mul(mu2[:, :], mu[:, :], mu[:, :])
        var = fs.tile([128, NQ], f32, name="var", tag="var")
        nc.vector.tensor_sub(var[:, :], msq[:, :], mu2[:, :])
        sd = fs.tile([128, NQ], f32, name="sd", tag="sd")
        nc.scalar.activation(sd[:, :], var[:, :], FT.Sqrt, bias=eps_t[:, :])
        rstd = fs.tile([128, NQ], f32, name="rstd", tag="rstd")
        nc.vector.reciprocal_approx_fast(rstd[:, :], sd[:, :])
        ln2 = [fs.tile([128, NQ], bf16, name=f"ln2{c}", tag=f"ln2{c}")
               for c in range(EC)]
        xm = fs.tile([128, NQ], f32, name="xm", tag="xm")
        for c in range(EC):
            nc.vector.tensor_sub(xm[:, :], h_sb[c][:, :], mu[:, :])
            xm2 = fs.tile([128, NQ], f32, name="xm2", tag="xm2")
            nc.vector.tensor_mul(xm2[:, :], xm[:, :], rstd[:, :])
            nc.vector.tensor_scalar(
                ln2[c][:, :], xm2[:, :],
                vecs[:, 2 * V_G2 + c:2 * V_G2 + c + 1],
                vecs[:, 2 * V_BETA2 + c:2 * V_BETA2 + c + 1],
                ALU.mult, ALU.add)
        z1 = [fs.tile([128, NQ], bf16, name=f"z1{c}", tag=f"z1{c}")
              for c in range(EC)]
        for qw in range(2):
            for fc in range(EC):
                p1 = fp.tile([128, 512], f32, name="ffn", tag="ffn")
                for ec in range(EC):
                    nc.tensor.matmul(
                        p1[:, :],
                        w_bf["W1"][:, E * ec + 128 * fc:E * ec + 128 * (fc + 1)],
                        ln2[ec][:, 512 * qw:512 * (qw + 1)],
                        start=(ec == 0), stop=(ec == EC - 1))
                nc.vector.tensor_scalar(z1[fc][:, 512 * qw:512 * (qw + 1)],
                                        p1[:, :],
                                        vecs[:, 2 * V_B1 + fc:2 * V_B1 + fc + 1],
                                        0.0, ALU.add, ALU.max)
        for qw in range(2):
            for fc in range(EC):
                p2 = fp.tile([128, 512], f32, name="ffn", tag="ffn")
                for ec in range(EC):
                    nc.tensor.matmul(
                        p2[:, :],
                        w_bf["W2"][:, E * ec + 128 * fc:E * ec + 128 * (fc + 1)],
                        z1[ec][:, 512 * qw:512 * (qw + 1)],
                        start=(ec == 0), stop=(ec == EC - 1))
                of = fs.tile([128, 512], f32, name="of", tag="of")
                nc.vector.scalar_tensor_tensor(
                    of[:, :], p2[:, :],
                    vecs[:, 2 * V_B2 + fc:2 * V_B2 + fc + 1],
                    h_sb[fc][:, 512 * qw:512 * (qw + 1)], ALU.add, ALU.add)
                nc.sync.dma_start(
                    outT_d[128 * fc:128 * (fc + 1), 512 * qw:512 * (qw + 1)],
                    of[:, :])

    for p in reversed(persist_pools):
        p.__exit__(None, None, None)


def build_nc():
    nc = bacc.Bacc(
        "TRN2",
        target_bir_lowering=False,
        debug=False,
        enable_asserts=False,
        num_devices=8,
    )
    ln1T_d = nc.dram_tensor("ln1T", [E, N], bf16, kind="ExternalInput").ap()
    xqT_d = nc.dram_tensor("xqT", [E, NQ], f32, kind="ExternalInput").ap()
    lgT_d = nc.dram_tensor("lgT", [N, NQ], bf16, kind="ExternalInput").ap()
    g2T_d = nc.dram_tensor("g2T", [N, NQ], bf16, kind="ExternalInput").ap()
    w_d = {
        name: nc.dram_tensor(name, [E, E], bf16, kind="ExternalInput").ap()
        for name in W_NAMES
    }
    vecs_d = nc.dram_tensor("vecs", [128, 10], f32, kind="ExternalInput").ap()
    ident_d = nc.dram_tensor("ident", [128, 128], bf16, kind="ExternalInput").ap()
    outT_d = nc.dram_tensor("outT", [E, NQ], f32, kind="ExternalOutput").ap()

    with tile.TileContext(nc) as tc:
        build_body(nc, tc, ln1T_d, xqT_d, lgT_d, g2T_d, w_d, vecs_d, ident_d,
                   outT_d)
    nc.compile()
    return nc


def host_shard(inputs):
    """Build the 8 per-core input maps (see module docstring)."""
    x = np.asarray(inputs["x"], np.float32)
    infl = np.asarray(inputs["influence_matrix"], np.float32)
    iw1 = float(np.asarray(inputs["iw1"]))
    ib1 = float(np.asarray(inputs["ib1"]))
    iw2 = float(np.asarray(inputs["iw2"]))
    ib2 = float(np.asarray(inputs["ib2"]))
    g1 = np.asarray(inputs["g1"], np.float32).reshape(E)
    beta1 = np.asarray(inputs["beta1"], np.float32).reshape(E)

    vec_list = ["g2", "beta2", "bo", "b1", "b2"]
    vecs_np = np.empty((128, 10), np.float32)
    for vi, nm in enumerate(vec_list):
        v = np.asarray(inputs[nm], np.float32).reshape(E)
        vecs_np[:, 2 * vi] = v[:128]
        vecs_np[:, 2 * vi + 1] = v[128:]

    ws = {}
    for n in W_NAMES:
        w = np.asarray(inputs[n], np.float32)
        if n == "Wq":
            w = w / math.sqrt(D)
        ws[n] = np.ascontiguousarray(w).astype(ml_dtypes.bfloat16)

    # host LN1 (input prep: pure function of inputs x, g1, beta1)
    mu = x.mean(axis=-1, keepdims=True)
    var = x.var(axis=-1, keepdims=True)
    ln1 = (x - mu) / np.sqrt(var + 1e-5) * g1 + beta1  # [B, N, E] f32

    ident_bf = np.eye(128, dtype=ml_dtypes.bfloat16)

    in_maps = []
    for core in range(8):
        b, qh = core // 2, core % 2
        qoff = qh * NQ
        ln1b = np.roll(ln1[b], -qoff, axis=0)          # [N, E]
        ln1T = np.ascontiguousarray(ln1b.T).astype(ml_dtypes.bfloat16)
        xqT = np.ascontiguousarray(x[b][qoff:qoff + NQ].T)  # [E, NQ] f32
        inf_slice = np.roll(infl[b][qoff:qoff + NQ, :], -qoff, axis=1)
        infT = inf_slice.T                              # [N(k), NQ(q)]
        lgT = np.ascontiguousarray(iw1 * infT + ib1).astype(ml_dtypes.bfloat16)
        g2T = np.ascontiguousarray(iw2 * infT + ib2).astype(ml_dtypes.bfloat16)
        m = {"ln1T": ln1T, "xqT": xqT, "lgT": lgT, "g2T": g2T,
             "vecs": vecs_np, "ident": ident_bf}
        m.update(ws)
        in_maps.append(m)
    return in_maps


_NC_CACHE = []


def kernel(**inputs):
    if not _NC_CACHE:
        _NC_CACHE.append(build_nc())
    nc = _NC_CACHE[0]
    in_maps = host_shard(inputs)
    res = run_bass_kernel_spmd(nc, in_maps, core_ids=list(range(8)))
    out = np.empty((B, N, E), np.float32)
    for core in range(8):
        b, qh = core // 2, core % 2
        out[b, qh * NQ:(qh + 1) * NQ, :] = np.asarray(
            res.results[core]["outT"], np.float32).T
    return out


# revision 8
# speedup vs baseline: 1.6544x; 1.5040x over previous
"""Graphormer layer on 8 TRN2 NeuronCores — v2 (ACT-bound redesign).

Sharding: core c handles batch b = c//2 and query-row half qh = c%2 (1024 q
rows). Transposed (feature-on-partition) layout throughout; host pre-rolls
the node axis per core so each core's own q rows sit first, and ships:
  - ln1T  : LayerNorm1(x) pre-computed, transposed, bf16
  - xqT   : x^T own-query columns (residual), f32
  - lgT   : (iw1*infl + ib1)^T per-core slice, bf16  (score bias)
  - g2T   : (iw2*infl + ib2)^T per-core slice, bf16  (post-softmax gate)
  - weights bf16 (Wq pre-scaled by 1/sqrt(D))

Device per core:
  B:  QT/KT/V projections from ln1T (bf16 matmuls)
  E:  per (qc, kc): S[128,2048] psum = LG preload (identity matmul) +
      packed per-head QK matmuls; e = exp(S) (ACT, the bottleneck ~118us);
      f = e*G2 (DVE 2x); Z += ones^T e, wv += V^T f (col-packed matmuls,
      emitted with one-iteration lag to keep PE FIFO stall-free)
  F:  attn = (wv/Z) @ Wo + bo + x ; LN2 + FFN + residual, store.
"""

import math

import numpy as np
import ml_dtypes

import concourse.bass as bass
import concourse.bacc as bacc
import concourse.mybir as mybir
import concourse.tile as tile
from concourse.bass_utils import run_bass_kernel_spmd

B, N, E, H, D = 4, 2048, 256, 8, 32
NQ = N // 2          # q rows per core
QC = 512             # q window
NKC = N // 128       # 16 k-chunks
EC = E // 128        # 2 feature chunks

f32 = mybir.dt.float32
bf16 = mybir.dt.bfloat16
FT = mybir.ActivationFunctionType
ALU = mybir.AluOpType

# vecs_sb column index: vec v, chunk c -> 2*v + c
V_G2, V_BETA2, V_BO, V_B1, V_B2 = range(5)
W_NAMES = ("Wq", "Wk", "Wv", "Wo", "W1", "W2")


def build_body(nc, tc, ln1T_d, xqT_d, lgT_d, g2T_d, w_d, vecs_d, ident_d,
               outT_d):
    persist_pools = []

    def ppool(name, space="SBUF"):
        p = tc.tile_pool(name=name, bufs=1, space=space)
        persist_pools.append(p)
        return p.__enter__()

    persist = ppool("persist")

    # ---- persistent SBUF ----
    qt = [persist.tile([128, NQ], bf16, name=f"qt{c}", tag=f"qt{c}") for c in range(EC)]
    kt = [persist.tile([128, N], bf16, name=f"kt{c}", tag=f"kt{c}") for c in range(EC)]
    xtq = [persist.tile([128, NQ], f32, name=f"xtq{c}", tag=f"xtq{c}") for c in range(EC)]
    v_sb = [persist.tile([128, E], bf16, name=f"v{k}", tag=f"v{k}") for k in range(NKC)]
    id_bf = persist.tile([128, 128], bf16, name="id_bf", tag="id_bf")
    w_bf = {n: persist.tile([128, 2 * E], bf16, name=f"w_{n}", tag=f"w_{n}")
            for n in W_NAMES}
    vecs = persist.tile([128, 10], f32, name="vecs", tag="vecs")
    ones = persist.tile([128, 128], f32, name="ones", tag="ones")
    ones_bf = persist.tile([128, 32], bf16, name="ones_bf", tag="ones_bf")
    ln1 = [persist.tile([128, N], bf16, name=f"ln1{c}", tag=f"ln1{c}") for c in range(EC)]
    h_sb = [persist.tile([128, NQ], f32, name=f"h{c}", tag=f"h{c}") for c in range(EC)]

    # ---- small loads ----
    for n in W_NAMES:
        for c in range(EC):
            nc.sync.dma_start(w_bf[n][:, E * c:E * (c + 1)],
                              w_d[n][128 * c:128 * (c + 1), :])
    nc.sync.dma_start(vecs[:, :], vecs_d[:, :])
    nc.sync.dma_start(id_bf[:, :], ident_d[:, :])
    for c in range(EC):
        nc.sync.dma_start(ln1[c][:, :], ln1T_d[128 * c:128 * (c + 1), :])
        nc.sync.dma_start(xtq[c][:, :], xqT_d[128 * c:128 * (c + 1), :])
    nc.vector.memset(ones[:, :], 1.0)
    nc.vector.memset(ones_bf[:, :], 1.0)
    eps_t = persist.tile([128, 1], f32, name="eps_t", tag="eps_t")
    nc.vector.memset(eps_t[:, :], 1e-5)
    zero_t = persist.tile([128, 1], f32, name="zero_t", tag="zero_t")
    nc.vector.memset(zero_t[:, :], 0.0)

    # ---- stage B: Q/K/V projections ----
    with tc.tile_pool(name="proj_psum", bufs=2, space="PSUM") as proj_psum:
        for fc in range(EC):
            for qw in range(NQ // 512):
                pq = proj_psum.tile([128, 512], f32, name="proj", tag="proj")
                for ec in range(EC):
                    nc.tensor.matmul(
                        pq[:, :],
                        w_bf["Wq"][:, E * ec + 128 * fc:E * ec + 128 * (fc + 1)],
                        ln1[ec][:, 512 * qw:512 * (qw + 1)],
                        start=(ec == 0), stop=(ec == EC - 1))
                nc.vector.tensor_copy(qt[fc][:, 512 * qw:512 * (qw + 1)], pq[:, :])
        for fc in range(EC):
            for kw in range(N // 512):
                pk = proj_psum.tile([128, 512], f32, name="proj", tag="proj")
                for ec in range(EC):
                    nc.tensor.matmul(
                        pk[:, :],
                        w_bf["Wk"][:, E * ec + 128 * fc:E * ec + 128 * (fc + 1)],
                        ln1[ec][:, 512 * kw:512 * (kw + 1)],
                        start=(ec == 0), stop=(ec == EC - 1))
                nc.vector.tensor_copy(kt[fc][:, 512 * kw:512 * (kw + 1)], pk[:, :])
        for k in range(NKC):
            pv = proj_psum.tile([128, E], f32, name="projv", tag="projv")
            for ec in range(EC):
                nc.tensor.matmul(
                    pv[:, :],
                    ln1[ec][:, 128 * k:128 * (k + 1)],
                    w_bf["Wv"][:, E * ec:E * (ec + 1)],
                    start=(ec == 0), stop=(ec == EC - 1))
            nc.vector.tensor_copy(v_sb[k][:, :], pv[:, :])

    # ---- stage E: attention ----
    # PSUM: S (4 banks) + wv (2) + z (2). S is single-buffered but consumed
    # in two 2-bank halves so next-iteration preloads only wait on the
    # matching half's exp. Z/wv matmuls are emitted one iteration late so
    # the PE FIFO never stalls behind ACT/DVE.
    with tc.tile_pool(name="score_psum", bufs=1, space="PSUM") as sp, \
         tc.tile_pool(name="acc_psum", bufs=1, space="PSUM") as ap_, \
         tc.tile_pool(name="ef_sbuf", bufs=5) as efp, \
         tc.tile_pool(name="g_sbuf", bufs=4) as gp, \
         tc.tile_pool(name="att_sbuf", bufs=2) as asb:
        # Two independent 2-bank score tiles so writes for iteration i+1 only
        # wait on the matching half's exp of iteration i (true pipelining).
        S2 = [sp.tile([128, 2 * QC], f32, name=f"S{h}", tag=f"S{h}")
              for h in range(2)]
        for qc in range(2):
            q0 = QC * qc
            wv_ps = [ap_.tile([128, QC], f32, name=f"wv{s}", tag=f"wv{s}")
                     for s in range(2)]
            z_ps = [ap_.tile([128, QC], f32, name=f"z{s}", tag=f"z{s}")
                    for s in range(2)]
            pending = []
            for kc in range(NKC):
                lg_t = gp.tile([128, QC], bf16, name="lg", tag="lg")
                g2_t = gp.tile([128, QC], bf16, name="g2", tag="g2")
                nc.sync.dma_start(lg_t[:, :], lgT_d[128 * kc:128 * (kc + 1),
                                                   q0:q0 + QC])
                nc.sync.dma_start(g2_t[:, :], g2T_d[128 * kc:128 * (kc + 1),
                                                    q0:q0 + QC])
                for hg in range(2):
                    c = hg  # head group hg covers heads 4hg..4hg+3 = chunk hg
                    g2b = g2_t[:, :].rearrange("p (o q) -> p o q", o=1)\
                        .broadcast_to([128, 2, QC])
                    ef = []
                    # half A (heads 0,1 of group) in S2[0]; half B in S2[1]
                    for half in range(2):
                        S = S2[half]
                        for jj in range(2):
                            nc.tensor.matmul(
                                S[:, QC * jj:QC * (jj + 1)],
                                id_bf[:, :], lg_t[:, :],
                                start=True, stop=False)
                        for jj in range(2):
                            j = 2 * half + jj
                            hh = 32 * j
                            nc.tensor.matmul(
                                S[:, QC * jj:QC * (jj + 1)],
                                kt[c][hh:hh + 32, 128 * kc:128 * (kc + 1)],
                                qt[c][hh:hh + 32, q0:q0 + QC],
                                start=False, stop=True,
                                skip_group_check=True, tile_position=(hh, 0))
                    # flush previous iteration's Z/wv matmuls (deps all met)
                    for fn in pending:
                        fn()
                    pending = []
                    for half in range(2):
                        S = S2[half]
                        e = efp.tile([128, 2 * QC], bf16, name="e", tag="e")
                        f = efp.tile([128, 2 * QC], bf16, name="f", tag="f")
                        nc.scalar.activation(e[:, :], S[:, :], FT.Exp,
                                             bias=zero_t[:, :])
                        nc.vector.tensor_tensor(
                            f[:, :].rearrange("p (o q) -> p o q", o=2),
                            e[:, :].rearrange("p (o q) -> p o q", o=2),
                            g2b, ALU.mult)
                        ef.append((e, f))

                    def mk(kc=kc, hg=hg, ef=ef):
                        def emit():
                            for j in range(4):
                                e = ef[j // 2][0]
                                nc.tensor.matmul(
                                    z_ps[hg][32 * j:32 * j + 32, :],
                                    ones_bf[:, :],
                                    e[:, QC * (j % 2):QC * (j % 2 + 1)],
                                    start=(kc == 0), stop=(kc == NKC - 1),
                                    skip_group_check=True,
                                    tile_position=(0, 32 * j))
                            for j in range(4):
                                f = ef[j // 2][1]
                                h = 4 * hg + j
                                nc.tensor.matmul(
                                    wv_ps[hg][32 * j:32 * j + 32, :],
                                    v_sb[kc][:, 32 * h:32 * h + 32],
                                    f[:, QC * (j % 2):QC * (j % 2 + 1)],
                                    start=(kc == 0), stop=(kc == NKC - 1),
                                    skip_group_check=True,
                                    tile_position=(0, 32 * j))
                        return emit
                    pending.append(mk())
            for fn in pending:
                fn()
            # normalize + Wo projection + bias + residual -> h
            on = []
            for s in range(2):
                zr = asb.tile([128, QC], f32, name=f"zr{s}", tag=f"zr{s}")
                nc.vector.reciprocal_approx_fast(zr[:, :], z_ps[s][:, :])
                o = asb.tile([128, QC], bf16, name=f"on{s}", tag=f"on{s}")
                nc.vector.tensor_mul(o[:, :], wv_ps[s][:, :], zr[:, :])
                on.append(o)
            for fc in range(EC):
                po = sp.tile([128, QC], f32, name="po", tag="S0")
                for ec in range(EC):
                    nc.tensor.matmul(
                        po[:, :],
                        w_bf["Wo"][:, E * ec + 128 * fc:E * ec + 128 * (fc + 1)],
                        on[ec][:, :],
                        start=(ec == 0), stop=(ec == EC - 1))
                nc.vector.scalar_tensor_tensor(
                    h_sb[fc][:, q0:q0 + QC], po[:, :],
                    vecs[:, 2 * V_BO + fc:2 * V_BO + fc + 1],
                    xtq[fc][:, q0:q0 + QC], ALU.add, ALU.add)

    # ---- stage F: LN2 + FFN + residual + store ----
    with tc.tile_pool(name="ln_psum2", bufs=2, space="PSUM") as ln_pp2, \
         tc.tile_pool(name="ffn_psum", bufs=2, space="PSUM") as fp, \
         tc.tile_pool(name="ffn_sbuf", bufs=2) as fs:
        # LN2 over partition dim (E = 2 chunks), full NQ width
        sq = [fs.tile([128, NQ], f32, name=f"sq{c}", tag=f"sq{c}") for c in range(EC)]
        for c in range(EC):
            nc.vector.tensor_mul(sq[c][:, :], h_sb[c][:, :], h_sb[c][:, :])
        p_s = [ln_pp2.tile([128, 512], f32, name="lnps", tag="lnps")
               for _ in range(2)]
        p_sq = [ln_pp2.tile([128, 512], f32, name="lnpsq", tag="lnpsq")
                for _ in range(2)]
        for w in range(2):
            for c in range(EC):
                nc.tensor.matmul(p_s[w][:, :], ones[:, :],
                                 h_sb[c][:, 512 * w:512 * (w + 1)],
                                 start=(c == 0), stop=(c == EC - 1))
            for c in range(EC):
                nc.tensor.matmul(p_sq[w][:, :], ones[:, :],
                                 sq[c][:, 512 * w:512 * (w + 1)],
                                 start=(c == 0), stop=(c == EC - 1))
        mu = fs.tile([128, NQ], f32, name="mu", tag="mu")
        msq = fs.tile([128, NQ], f32, name="msq", tag="msq")
        for w in range(2):
            nc.vector.tensor_scalar_mul(mu[:, 512 * w:512 * (w + 1)],
                                        p_s[w][:, :], 1.0 / E)
            nc.vector.tensor_scalar_mul(msq[:, 512 * w:512 * (w + 1)],
                                        p_sq[w][:, :], 1.0 / E)
        mu2 = fs.tile([128, NQ], f32, name="mu2", tag="mu2")
        nc.vector.tensor_mul(mu2[:, :], mu[:, :], mu[:, :])
        var = fs.tile([128, NQ], f32, name="var", tag="var")
        nc.vector.tensor_sub(var[:, :], msq[:, :], mu2[:, :])
        sd = fs.tile([128, NQ], f32, name="sd", tag="sd")
        nc.scalar.activation(sd[:, :], var[:, :], FT.Sqrt, bias=eps_t[:, :])
        rstd = fs.tile([128, NQ], f32, name="rstd", tag="rstd")
        nc.vector.reciprocal_approx_fast(rstd[:, :], sd[:, :])
        ln2 = [fs.tile([128, NQ], bf16, name=f"ln2{c}", tag=f"ln2{c}")
               for c in range(EC)]
        xm = fs.tile([128, NQ], f32, name="xm", tag="xm")
        for c in range(EC):
            nc.vector.tensor_sub(xm[:, :], h_sb[c][:, :], mu[:, :])
            xm2 = fs.tile([128, NQ], f32, name="xm2", tag="xm2")
            nc.vector.tensor_mul(xm2[:, :], xm[:, :], rstd[:, :])
            nc.vector.tensor_scalar(
                ln2[c][:, :], xm2[:, :],
                vecs[:, 2 * V_G2 + c:2 * V_G2 + c + 1],
                vecs[:, 2 * V_BETA2 + c:2 * V_BETA2 + c + 1],
                ALU.mult, ALU.add)
        z1 = [fs.tile([128, NQ], bf16, name=f"z1{c}", tag=f"z1{c}")
              for c in range(EC)]
        for qw in range(2):
            for fc in range(EC):
                p1 = fp.tile([128, 512], f32, name="ffn", tag="ffn")
                for ec in range(EC):
                    nc.tensor.matmul(
                        p1[:, :],
                        w_bf["W1"][:, E * ec + 128 * fc:E * ec + 128 * (fc + 1)],
                        ln2[ec][:, 512 * qw:512 * (qw + 1)],
                        start=(ec == 0), stop=(ec == EC - 1))
                nc.vector.tensor_scalar(z1[fc][:, 512 * qw:512 * (qw + 1)],
                                        p1[:, :],
                                        vecs[:, 2 * V_B1 + fc:2 * V_B1 + fc + 1],
                                        0.0, ALU.add, ALU.max)
        for qw in range(2):
            for fc in range(EC):
                p2 = fp.tile([128, 512], f32, name="ffn", tag="ffn")
                for ec in range(EC):
                    nc.tensor.matmul(
                        p2[:, :],
                        w_bf["W2"][:, E * ec + 128 * fc:E * ec + 128 * (fc + 1)],
                        z1[ec][:, 512 * qw:512 * (qw + 1)],
                        start=(ec == 0), stop=(ec == EC - 1))
                of = fs.tile([128, 512], f32, name="of", tag="of")
                nc.vector.scalar_tensor_tensor(
                    of[:, :], p2[:, :],
                    vecs[:, 2 * V_B2 + fc:2 * V_B2 + fc + 1],
                    h_sb[fc][:, 512 * qw:512 * (qw + 1)], ALU.add, ALU.add)
                nc.sync.dma_start(
                    outT_d[128 * fc:128 * (fc + 1), 512 * qw:512 * (qw + 1)],
                    of[:, :])

    for p in reversed(persist_pools):
        p.__exit__(None, None, None)


def build_nc():
    nc = bacc.Bacc(
        "TRN2",
        target_bir_lowering=False,
        debug=False,
        enable_asserts=False,
        num_devices=8,
    )
    ln1T_d = nc.dram_tensor("ln1T", [E, N], bf16, kind="ExternalInput").ap()
    xqT_d = nc.dram_tensor("xqT", [E, NQ], f32, kind="ExternalInput").ap()
    lgT_d = nc.dram_tensor("lgT", [N, NQ], bf16, kind="ExternalInput").ap()
    g2T_d = nc.dram_tensor("g2T", [N, NQ], bf16, kind="ExternalInput").ap()
    w_d = {
        name: nc.dram_tensor(name, [E, E], bf16, kind="ExternalInput").ap()
        for name in W_NAMES
    }
    vecs_d = nc.dram_tensor("vecs", [128, 10], f32, kind="ExternalInput").ap()
    ident_d = nc.dram_tensor("ident", [128, 128], bf16, kind="ExternalInput").ap()
    outT_d = nc.dram_tensor("outT", [E, NQ], f32, kind="ExternalOutput").ap()

    with tile.TileContext(nc) as tc:
        build_body(nc, tc, ln1T_d, xqT_d, lgT_d, g2T_d, w_d, vecs_d, ident_d,
                   outT_d)
    nc.compile()
    return nc


def host_shard(inputs):
    """Build the 8 per-core input maps (see module docstring)."""
    x = np.asarray(inputs["x"], np.float32)
    infl = np.asarray(inputs["influence_matrix"], np.float32)
    iw1 = float(np.asarray(inputs["iw1"]))
    ib1 = float(np.asarray(inputs["ib1"]))
    iw2 = float(np.asarray(inputs["iw2"]))
    ib2 = float(np.asarray(inputs["ib2"]))
    g1 = np.asarray(inputs["g1"], np.float32).reshape(E)
    beta1 = np.asarray(inputs["beta1"], np.float32).reshape(E)

    vec_list = ["g2", "beta2", "bo", "b1", "b2"]
    vecs_np = np.empty((128, 10), np.float32)
    for vi, nm in enumerate(vec_list):
        v = np.asarray(inputs[nm], np.float32).reshape(E)
        vecs_np[:, 2 * vi] = v[:128]
        vecs_np[:, 2 * vi + 1] = v[128:]

    ws = {}
    for n in W_NAMES:
        w = np.asarray(inputs[n], np.float32)
        if n == "Wq":
            w = w / math.sqrt(D)
        ws[n] = np.ascontiguousarray(w).astype(ml_dtypes.bfloat16)

    # host LN1 (input prep: pure function of inputs x, g1, beta1)
    mu = x.mean(axis=-1, keepdims=True)
    var = x.var(axis=-1, keepdims=True)
    ln1 = (x - mu) / np.sqrt(var + 1e-5) * g1 + beta1  # [B, N, E] f32

    ident_bf = np.eye(128, dtype=ml_dtypes.bfloat16)

    in_maps = []
    for core in range(8):
        b, qh = core // 2, core % 2
        qoff = qh * NQ
        ln1b = np.roll(ln1[b], -qoff, axis=0)          # [N, E]
        ln1T = np.ascontiguousarray(ln1b.T).astype(ml_dtypes.bfloat16)
        xqT = np.ascontiguousarray(x[b][qoff:qoff + NQ].T)  # [E, NQ] f32
        inf_slice = np.roll(infl[b][qoff:qoff + NQ, :], -qoff, axis=1)
        infT = inf_slice.T                              # [N(k), NQ(q)]
        lgT = np.ascontiguousarray(iw1 * infT + ib1).astype(ml_dtypes.bfloat16)
        g2T = np.ascontiguousarray(iw2 * infT + ib2).astype(ml_dtypes.bfloat16)
        m = {"ln1T": ln1T, "xqT": xqT, "lgT": lgT, "g2T": g2T,
             "vecs": vecs_np, "ident": ident_bf}
        m.update(ws)
        in_maps.append(m)
    return in_maps


_NC_CACHE = []


def kernel(**inputs):
    if not _NC_CACHE:
        _NC_CACHE.append(build_nc())
    nc = _NC_CACHE[0]
    in_maps = host_shard(inputs)
    res = run_bass_kernel_spmd(nc, in_maps, core_ids=list(range(8)))
    out = np.empty((B, N, E), np.float32)
    for core in range(8):
        b, qh = core // 2, core % 2
        out[b, qh * NQ:(qh + 1) * NQ, :] = np.asarray(
            res.results[core]["outT"], np.float32).T
    return out


# revision 9
# speedup vs baseline: 1.7278x; 1.0444x over previous
"""Graphormer layer on 8 TRN2 NeuronCores — v4 (ACT-bound, packed DMA).

Sharding: core c handles batch b = c//2 and query-row half qh = c%2 (1024 q
rows). Transposed (feature-on-partition) layout throughout; host pre-rolls
the node axis per core so each core's own q rows sit first, and ships:
  - ln1b  : LayerNorm1(x) pre-computed, transposed, bf16 (packed chunks)
  - xqb   : x^T own-query columns (residual), f32 (packed chunks)
  - lgg2  : [(iw1*u+ib1) | (iw2*u+ib2)]^T per-core slice, bf16, fused
  - wblob : all six weight matrices (Wq pre-scaled 1/sqrt(D)) + identity

Device per core:
  B:  QT/KT/V projections from ln1 (bf16 matmuls; kt casts on ScalarE)
  E:  per (qc, kc, hg): two 2-bank score tiles S2[half] get LG preload
      (identity matmul) + 2 row-packed QK matmuls each; exp on [128,1024]
      (ACT = bottleneck ~147us); f = e*G2 (DVE 2x); Z/wv via 4-way
      col-packed matmuls emitted with one-iteration lag (PE FIFO never
      stalls on ACT/DVE).
  F:  attn = (wv/Z) @ Wo + bo + x ; LN2 + FFN + residual, store.
"""

import math

import numpy as np
import ml_dtypes

import concourse.bass as bass
import concourse.bacc as bacc
import concourse.mybir as mybir
import concourse.tile as tile
from concourse.bass_utils import run_bass_kernel_spmd

B, N, E, H, D = 4, 2048, 256, 8, 32
NQ = N // 2          # q rows per core
QC = 512             # q window
NKC = N // 128       # 16 k-chunks
EC = E // 128        # 2 feature chunks

f32 = mybir.dt.float32
bf16 = mybir.dt.bfloat16
FT = mybir.ActivationFunctionType
ALU = mybir.AluOpType

V_G2, V_BETA2, V_BO, V_B1, V_B2 = range(5)
W_NAMES = ("Wq", "Wk", "Wv", "Wo", "W1", "W2")


def build_body(nc, tc, ln1b_d, xqb_d, lgg2_d, wblob_d, vecs_d, outT_d):
    persist_pools = []

    def ppool(name, space="SBUF"):
        p = tc.tile_pool(name=name, bufs=1, space=space)
        persist_pools.append(p)
        return p.__enter__()

    persist = ppool("persist")

    # ---- persistent SBUF ----
    qt = [persist.tile([128, NQ], bf16, name=f"qt{c}", tag=f"qt{c}") for c in range(EC)]
    kt = [persist.tile([128, N], bf16, name=f"kt{c}", tag=f"kt{c}") for c in range(EC)]
    v_sb = [persist.tile([128, E], bf16, name=f"v{k}", tag=f"v{k}") for k in range(NKC)]
    wblob = persist.tile([128, 3200], bf16, name="wblob", tag="wblob")
    lnb = persist.tile([128, 2 * N], bf16, name="lnb", tag="lnb")
    xqb = persist.tile([128, 2 * NQ], f32, name="xqb", tag="xqb")
    vecs = persist.tile([128, 10], f32, name="vecs", tag="vecs")
    ones = persist.tile([128, 128], f32, name="ones", tag="ones")
    ones_bf = persist.tile([128, 32], bf16, name="ones_bf", tag="ones_bf")
    h_sb = [persist.tile([128, NQ], f32, name=f"h{c}", tag=f"h{c}") for c in range(EC)]

    w_bf = {n: wblob[:, 512 * i:512 * (i + 1)] for i, n in enumerate(W_NAMES)}
    id_bf = wblob[:, 3072:3200]
    ln1 = [lnb[:, N * c:N * (c + 1)] for c in range(EC)]
    xtq = [xqb[:, NQ * c:NQ * (c + 1)] for c in range(EC)]

    # ---- packed loads ----
    nc.sync.dma_start(wblob[:, :], wblob_d[:, :])
    nc.sync.dma_start(lnb[:, :], ln1b_d[:, :])
    nc.sync.dma_start(xqb[:, :], xqb_d[:, :])
    nc.sync.dma_start(vecs[:, :], vecs_d[:, :])
    nc.vector.memset(ones[:, :], 1.0)
    nc.vector.memset(ones_bf[:, :], 1.0)
    eps_t = persist.tile([128, 1], f32, name="eps_t", tag="eps_t")
    nc.vector.memset(eps_t[:, :], 1e-5)
    zero_t = persist.tile([128, 1], f32, name="zero_t", tag="zero_t")
    nc.vector.memset(zero_t[:, :], 0.0)

    # ---- stage B: Q/K/V projections ----
    with tc.tile_pool(name="proj_psum", bufs=4, space="PSUM") as proj_psum:
        for fc in range(EC):
            for qw in range(NQ // 512):
                pq = proj_psum.tile([128, 512], f32, name="proj", tag="proj")
                for ec in range(EC):
                    nc.tensor.matmul(
                        pq[:, :],
                        w_bf["Wq"][:, E * ec + 128 * fc:E * ec + 128 * (fc + 1)],
                        ln1[ec][:, 512 * qw:512 * (qw + 1)],
                        start=(ec == 0), stop=(ec == EC - 1))
                nc.vector.tensor_copy(qt[fc][:, 512 * qw:512 * (qw + 1)], pq[:, :])
        for fc in range(EC):
            for kw in range(N // 512):
                pk = proj_psum.tile([128, 512], f32, name="proj", tag="proj")
                for ec in range(EC):
                    nc.tensor.matmul(
                        pk[:, :],
                        w_bf["Wk"][:, E * ec + 128 * fc:E * ec + 128 * (fc + 1)],
                        ln1[ec][:, 512 * kw:512 * (kw + 1)],
                        start=(ec == 0), stop=(ec == EC - 1))
                nc.scalar.copy(kt[fc][:, 512 * kw:512 * (kw + 1)], pk[:, :])
        for k in range(NKC):
            pv = proj_psum.tile([128, E], f32, name="projv", tag="projv")
            for ec in range(EC):
                nc.tensor.matmul(
                    pv[:, :],
                    ln1[ec][:, 128 * k:128 * (k + 1)],
                    w_bf["Wv"][:, E * ec:E * (ec + 1)],
                    start=(ec == 0), stop=(ec == EC - 1))
            nc.vector.tensor_copy(v_sb[k][:, :], pv[:, :])

    # ---- stage E: attention ----
    with tc.tile_pool(name="score_psum", bufs=1, space="PSUM") as sp, \
         tc.tile_pool(name="acc_psum", bufs=1, space="PSUM") as ap_, \
         tc.tile_pool(name="ef_sbuf", bufs=5) as efp, \
         tc.tile_pool(name="g_sbuf", bufs=4) as gp, \
         tc.tile_pool(name="att_sbuf", bufs=2) as asb:
        # Two independent 2-bank score tiles so writes for iteration i+1 only
        # wait on the matching half's exp of iteration i (true pipelining).
        S2 = [sp.tile([128, 2 * QC], f32, name=f"S{h}", tag=f"S{h}")
              for h in range(2)]
        for qc in range(2):
            q0 = QC * qc
            wv_ps = [ap_.tile([128, QC], f32, name=f"wv{s}", tag=f"wv{s}")
                     for s in range(2)]
            z_ps = [ap_.tile([128, QC], f32, name=f"z{s}", tag=f"z{s}")
                    for s in range(2)]
            pending = []
            for kc in range(NKC):
                gt = gp.tile([128, 2 * QC], bf16, name="lgg2", tag="lgg2")
                nc.sync.dma_start(gt[:, :],
                                  lgg2_d[128 * kc:128 * (kc + 1),
                                         2 * QC * qc:2 * QC * (qc + 1)])
                lg_t = gt[:, 0:QC]
                g2_t = gt[:, QC:2 * QC]
                for hg in range(2):
                    c = hg  # head group hg covers heads 4hg..4hg+3 = chunk hg
                    g2b = g2_t.rearrange("p (o q) -> p o q", o=1)\
                        .broadcast_to([128, 2, QC])
                    ef = []
                    # half A (heads 0,1 of group) in S2[0]; half B in S2[1]
                    for half in range(2):
                        S = S2[half]
                        for jj in range(2):
                            nc.tensor.matmul(
                                S[:, QC * jj:QC * (jj + 1)],
                                id_bf, lg_t,
                                start=True, stop=False)
                        for jj in range(2):
                            j = 2 * half + jj
                            hh = 32 * j
                            nc.tensor.matmul(
                                S[:, QC * jj:QC * (jj + 1)],
                                kt[c][hh:hh + 32, 128 * kc:128 * (kc + 1)],
                                qt[c][hh:hh + 32, q0:q0 + QC],
                                start=False, stop=True,
                                skip_group_check=True, tile_position=(hh, 0))
                    # flush previous iteration's Z/wv matmuls (deps all met)
                    for fn in pending:
                        fn()
                    pending = []
                    for half in range(2):
                        S = S2[half]
                        e = efp.tile([128, 2 * QC], bf16, name="e", tag="e")
                        f = efp.tile([128, 2 * QC], bf16, name="f", tag="f")
                        nc.scalar.activation(e[:, :], S[:, :], FT.Exp,
                                             bias=zero_t[:, :])
                        nc.vector.tensor_tensor(
                            f[:, :].rearrange("p (o q) -> p o q", o=2),
                            e[:, :].rearrange("p (o q) -> p o q", o=2),
                            g2b, ALU.mult)
                        ef.append((e, f))

                    def mk(kc=kc, hg=hg, ef=ef):
                        def emit():
                            for j in range(4):
                                e = ef[j // 2][0]
                                nc.tensor.matmul(
                                    z_ps[hg][32 * j:32 * j + 32, :],
                                    ones_bf[:, :],
                                    e[:, QC * (j % 2):QC * (j % 2 + 1)],
                                    start=(kc == 0), stop=(kc == NKC - 1),
                                    skip_group_check=True,
                                    tile_position=(0, 32 * j))
                            for j in range(4):
                                f = ef[j // 2][1]
                                h = 4 * hg + j
                                nc.tensor.matmul(
                                    wv_ps[hg][32 * j:32 * j + 32, :],
                                    v_sb[kc][:, 32 * h:32 * h + 32],
                                    f[:, QC * (j % 2):QC * (j % 2 + 1)],
                                    start=(kc == 0), stop=(kc == NKC - 1),
                                    skip_group_check=True,
                                    tile_position=(0, 32 * j))
                        return emit
                    pending.append(mk())
            for fn in pending:
                fn()
            # normalize + Wo projection + bias + residual -> h
            on = []
            for s in range(2):
                zr = asb.tile([128, QC], f32, name=f"zr{s}", tag=f"zr{s}")
                nc.vector.reciprocal_approx_fast(zr[:, :], z_ps[s][:, :])
                o = asb.tile([128, QC], bf16, name=f"on{s}", tag=f"on{s}")
                nc.vector.tensor_mul(o[:, :], wv_ps[s][:, :], zr[:, :])
                on.append(o)
            for fc in range(EC):
                po = sp.tile([128, QC], f32, name="po", tag="S0")
                for ec in range(EC):
                    nc.tensor.matmul(
                        po[:, :],
                        w_bf["Wo"][:, E * ec + 128 * fc:E * ec + 128 * (fc + 1)],
                        on[ec][:, :],
                        start=(ec == 0), stop=(ec == EC - 1))
                nc.vector.scalar_tensor_tensor(
                    h_sb[fc][:, q0:q0 + QC], po[:, :],
                    vecs[:, 2 * V_BO + fc:2 * V_BO + fc + 1],
                    xtq[fc][:, q0:q0 + QC], ALU.add, ALU.add)

    # ---- stage F: LN2 + FFN + residual + store ----
    with tc.tile_pool(name="ln_psum2", bufs=2, space="PSUM") as ln_pp2, \
         tc.tile_pool(name="ffn_psum", bufs=2, space="PSUM") as fp, \
         tc.tile_pool(name="ffn_sbuf", bufs=2) as fs:
        # mean
        p_s = [ln_pp2.tile([128, 512], f32, name="lnps", tag="lnps")
               for _ in range(2)]
        for w in range(2):
            for c in range(EC):
                nc.tensor.matmul(p_s[w][:, :], ones[:, :],
                                 h_sb[c][:, 512 * w:512 * (w + 1)],
                                 start=(c == 0), stop=(c == EC - 1))
        mu = fs.tile([128, NQ], f32, name="mu", tag="mu")
        for w in range(2):
            nc.vector.tensor_scalar_mul(mu[:, 512 * w:512 * (w + 1)],
                                        p_s[w][:, :], 1.0 / E)
        # xm = h - mu ; var = mean(xm^2)
        xm = [fs.tile([128, NQ], f32, name=f"xm{c}", tag=f"xm{c}")
              for c in range(EC)]
        sqx = fs.tile([128, NQ], f32, name="sqx", tag="sqx")
        p_sq = [ln_pp2.tile([128, 512], f32, name="lnpsq", tag="lnpsq")
                for _ in range(2)]
        for c in range(EC):
            nc.vector.tensor_sub(xm[c][:, :], h_sb[c][:, :], mu[:, :])
        for c in range(EC):
            nc.vector.tensor_mul(sqx[:, :], xm[c][:, :], xm[c][:, :])
            for w in range(2):
                nc.tensor.matmul(p_sq[w][:, :], ones[:, :],
                                 sqx[:, 512 * w:512 * (w + 1)],
                                 start=(c == 0), stop=(c == EC - 1))
        var = fs.tile([128, NQ], f32, name="var", tag="var")
        for w in range(2):
            nc.vector.tensor_scalar_mul(var[:, 512 * w:512 * (w + 1)],
                                        p_sq[w][:, :], 1.0 / E)
        sd = fs.tile([128, NQ], f32, name="sd", tag="sd")
        nc.scalar.activation(sd[:, :], var[:, :], FT.Sqrt, bias=eps_t[:, :])
        rstd = fs.tile([128, NQ], f32, name="rstd", tag="rstd")
        nc.vector.reciprocal_approx_fast(rstd[:, :], sd[:, :])
        ln2 = [fs.tile([128, NQ], bf16, name=f"ln2{c}", tag=f"ln2{c}")
               for c in range(EC)]
        for c in range(EC):
            xm2 = fs.tile([128, NQ], f32, name="xm2", tag="xm2")
            nc.vector.tensor_mul(xm2[:, :], xm[c][:, :], rstd[:, :])
            nc.vector.tensor_scalar(
                ln2[c][:, :], xm2[:, :],
                vecs[:, 2 * V_G2 + c:2 * V_G2 + c + 1],
                vecs[:, 2 * V_BETA2 + c:2 * V_BETA2 + c + 1],
                ALU.mult, ALU.add)
        z1 = [fs.tile([128, NQ], bf16, name=f"z1{c}", tag=f"z1{c}")
              for c in range(EC)]
        for qw in range(2):
            for fc in range(EC):
                p1 = fp.tile([128, 512], f32, name="ffn", tag="ffn")
                for ec in range(EC):
                    nc.tensor.matmul(
                        p1[:, :],
                        w_bf["W1"][:, E * ec + 128 * fc:E * ec + 128 * (fc + 1)],
                        ln2[ec][:, 512 * qw:512 * (qw + 1)],
                        start=(ec == 0), stop=(ec == EC - 1))
                nc.vector.tensor_scalar(z1[fc][:, 512 * qw:512 * (qw + 1)],
                                        p1[:, :],
                                        vecs[:, 2 * V_B1 + fc:2 * V_B1 + fc + 1],
                                        0.0, ALU.add, ALU.max)
        for qw in range(2):
            for fc in range(EC):
                p2 = fp.tile([128, 512], f32, name="ffn", tag="ffn")
                for ec in range(EC):
                    nc.tensor.matmul(
                        p2[:, :],
                        w_bf["W2"][:, E * ec + 128 * fc:E * ec + 128 * (fc + 1)],
                        z1[ec][:, 512 * qw:512 * (qw + 1)],
                        start=(ec == 0), stop=(ec == EC - 1))
                of = fs.tile([128, 512], f32, name="of", tag="of")
                nc.vector.scalar_tensor_tensor(
                    of[:, :], p2[:, :],
                    vecs[:, 2 * V_B2 + fc:2 * V_B2 + fc + 1],
                    h_sb[fc][:, 512 * qw:512 * (qw + 1)], ALU.add, ALU.add)
                nc.sync.dma_start(
                    outT_d[128 * fc:128 * (fc + 1), 512 * qw:512 * (qw + 1)],
                    of[:, :])

    for p in reversed(persist_pools):
        p.__exit__(None, None, None)


def build_nc():
    nc = bacc.Bacc(
        "TRN2",
        target_bir_lowering=False,
        debug=False,
        enable_asserts=False,
        num_devices=8,
    )
    ln1b_d = nc.dram_tensor("ln1b", [128, 2 * N], bf16, kind="ExternalInput").ap()
    xqb_d = nc.dram_tensor("xqb", [128, 2 * NQ], f32, kind="ExternalInput").ap()
    lgg2_d = nc.dram_tensor("lgg2", [N, 2 * NQ], bf16, kind="ExternalInput").ap()
    wblob_d = nc.dram_tensor("wblob", [128, 3200], bf16, kind="ExternalInput").ap()
    vecs_d = nc.dram_tensor("vecs", [128, 10], f32, kind="ExternalInput").ap()
    outT_d = nc.dram_tensor("outT", [E, NQ], f32, kind="ExternalOutput").ap()

    with tile.TileContext(nc) as tc:
        build_body(nc, tc, ln1b_d, xqb_d, lgg2_d, wblob_d, vecs_d, outT_d)
    nc.compile()
    return nc


def host_shard(inputs):
    """Build the 8 per-core input maps (see module docstring)."""
    x = np.asarray(inputs["x"], np.float32)
    infl = np.asarray(inputs["influence_matrix"], np.float32)
    iw1 = float(np.asarray(inputs["iw1"]))
    ib1 = float(np.asarray(inputs["ib1"]))
    iw2 = float(np.asarray(inputs["iw2"]))
    ib2 = float(np.asarray(inputs["ib2"]))
    g1 = np.asarray(inputs["g1"], np.float32).reshape(E)
    beta1 = np.asarray(inputs["beta1"], np.float32).reshape(E)

    vec_list = ["g2", "beta2", "bo", "b1", "b2"]
    vecs_np = np.empty((128, 10), np.float32)
    for vi, nm in enumerate(vec_list):
        v = np.asarray(inputs[nm], np.float32).reshape(E)
        vecs_np[:, 2 * vi] = v[:128]
        vecs_np[:, 2 * vi + 1] = v[128:]

    # weight blob: 6 matrices as [128, 512] (chunk-major cols) + identity
    wblob = np.zeros((128, 3200), np.float32)
    for i, n in enumerate(W_NAMES):
        w = np.asarray(inputs[n], np.float32)
        if n == "Wq":
            w = w / math.sqrt(D)
        for c in range(EC):
            wblob[:, 512 * i + E * c:512 * i + E * (c + 1)] = w[128 * c:128 * (c + 1), :]
    wblob[:, 3072:3200] = np.eye(128, dtype=np.float32)
    wblob = wblob.astype(ml_dtypes.bfloat16)

    # host LN1 (input prep: pure function of inputs x, g1, beta1)
    mu = x.mean(axis=-1, keepdims=True)
    var = x.var(axis=-1, keepdims=True)
    ln1 = (x - mu) / np.sqrt(var + 1e-5) * g1 + beta1  # [B, N, E] f32

    in_maps = []
    for core in range(8):
        b, qh = core // 2, core % 2
        qoff = qh * NQ
        ln1b = np.roll(ln1[b], -qoff, axis=0)          # [N, E]
        ln1T = ln1b.T                                  # [E, N]
        ln1_pack = np.empty((128, 2 * N), np.float32)
        for c in range(EC):
            ln1_pack[:, N * c:N * (c + 1)] = ln1T[128 * c:128 * (c + 1), :]
        xqT = x[b][qoff:qoff + NQ].T                   # [E, NQ]
        xq_pack = np.empty((128, 2 * NQ), np.float32)
        for c in range(EC):
            xq_pack[:, NQ * c:NQ * (c + 1)] = xqT[128 * c:128 * (c + 1), :]
        inf_slice = np.roll(infl[b][qoff:qoff + NQ, :], -qoff, axis=1)
        infT = inf_slice.T                              # [N(k), NQ(q)]
        lgg2 = np.empty((N, 2 * NQ), np.float32)
        for q in range(2):
            lgg2[:, 1024 * q:1024 * q + 512] = iw1 * infT[:, 512 * q:512 * (q + 1)] + ib1
            lgg2[:, 1024 * q + 512:1024 * (q + 1)] = iw2 * infT[:, 512 * q:512 * (q + 1)] + ib2
        m = {"ln1b": ln1_pack.astype(ml_dtypes.bfloat16),
             "xqb": np.ascontiguousarray(xq_pack),
             "lgg2": lgg2.astype(ml_dtypes.bfloat16),
             "wblob": wblob, "vecs": vecs_np}
        in_maps.append(m)
    return in_maps


_NC_CACHE = []


def kernel(**inputs):
    if not _NC_CACHE:
        _NC_CACHE.append(build_nc())
    nc = _NC_CACHE[0]
    in_maps = host_shard(inputs)
    res = run_bass_kernel_spmd(nc, in_maps, core_ids=list(range(8)))
    out = np.empty((B, N, E), np.float32)
    for core in range(8):
        b, qh = core // 2, core % 2
        out[b, qh * NQ:(qh + 1) * NQ, :] = np.asarray(
            res.results[core]["outT"], np.float32).T
    return out


# revision 12
# speedup vs baseline: 1.7497x; 1.0127x over previous
"""Graphormer layer on 8 TRN2 NeuronCores — v4 (ACT-bound, packed DMA).

Sharding: core c handles batch b = c//2 and query-row half qh = c%2 (1024 q
rows). Transposed (feature-on-partition) layout throughout; host pre-rolls
the node axis per core so each core's own q rows sit first, and ships:
  - ln1b  : LayerNorm1(x) pre-computed, transposed, bf16 (packed chunks)
  - xqb   : x^T own-query columns (residual), f32 (packed chunks)
  - lgg2  : [(iw1*u+ib1) | (iw2*u+ib2)]^T per-core slice, bf16, fused
  - wblob : all six weight matrices (Wq pre-scaled 1/sqrt(D)) + identity

Device per core:
  B:  QT/KT/V projections from ln1 (bf16 matmuls; kt casts on ScalarE)
  E:  per (qc, kc, hg): two 2-bank score tiles S2[half] get LG preload
      (identity matmul) + 2 row-packed QK matmuls each; exp on [128,1024]
      (ACT = bottleneck ~147us); f = e*G2 (DVE 2x); Z/wv via 4-way
      col-packed matmuls emitted with one-iteration lag (PE FIFO never
      stalls on ACT/DVE).
  F:  attn = (wv/Z) @ Wo + bo + x ; LN2 + FFN + residual, store.
"""

import math

import numpy as np
import ml_dtypes

import concourse.bass as bass
import concourse.bacc as bacc
import concourse.mybir as mybir
import concourse.tile as tile
from concourse.bass_utils import run_bass_kernel_spmd

B, N, E, H, D = 4, 2048, 256, 8, 32
NQ = N // 2          # q rows per core
QC = 512             # q window
NKC = N // 128       # 16 k-chunks
EC = E // 128        # 2 feature chunks

f32 = mybir.dt.float32
bf16 = mybir.dt.bfloat16
FT = mybir.ActivationFunctionType
ALU = mybir.AluOpType

V_G2, V_BETA2, V_BO, V_B1, V_B2 = range(5)
W_NAMES = ("Wq", "Wk", "Wv", "Wo", "W1", "W2")


def build_body(nc, tc, ln1b_d, xqb_d, lgg2_d, wblob_d, vecs_d, outT_d):
    persist_pools = []

    def ppool(name, space="SBUF"):
        p = tc.tile_pool(name=name, bufs=1, space=space)
        persist_pools.append(p)
        return p.__enter__()

    persist = ppool("persist")

    # ---- persistent SBUF ----
    qt = [persist.tile([128, NQ], bf16, name=f"qt{c}", tag=f"qt{c}") for c in range(EC)]
    kt = [persist.tile([128, N], bf16, name=f"kt{c}", tag=f"kt{c}") for c in range(EC)]
    v_sb = [persist.tile([128, E], bf16, name=f"v{k}", tag=f"v{k}") for k in range(NKC)]
    wblob = persist.tile([128, 3200], bf16, name="wblob", tag="wblob")
    lnb = persist.tile([128, 2 * N], bf16, name="lnb", tag="lnb")
    xqb = persist.tile([128, 2 * NQ], f32, name="xqb", tag="xqb")
    vecs = persist.tile([128, 10], f32, name="vecs", tag="vecs")
    ones = persist.tile([128, 128], f32, name="ones", tag="ones")
    ones_bf = persist.tile([128, 32], bf16, name="ones_bf", tag="ones_bf")
    h_sb = [persist.tile([128, NQ], f32, name=f"h{c}", tag=f"h{c}") for c in range(EC)]

    w_bf = {n: wblob[:, 512 * i:512 * (i + 1)] for i, n in enumerate(W_NAMES)}
    id_bf = wblob[:, 3072:3200]
    ln1 = [lnb[:, N * c:N * (c + 1)] for c in range(EC)]
    xtq = [xqb[:, NQ * c:NQ * (c + 1)] for c in range(EC)]

    # ---- packed loads ----
    nc.sync.dma_start(wblob[:, :], wblob_d[:, :])
    nc.sync.dma_start(lnb[:, :], ln1b_d[:, :])
    nc.sync.dma_start(xqb[:, :], xqb_d[:, :])
    nc.sync.dma_start(vecs[:, :], vecs_d[:, :])
    nc.vector.memset(ones[:, :], 1.0)
    nc.vector.memset(ones_bf[:, :], 1.0)
    eps_t = persist.tile([128, 1], f32, name="eps_t", tag="eps_t")
    nc.vector.memset(eps_t[:, :], 1e-5)
    zero_t = persist.tile([128, 1], f32, name="zero_t", tag="zero_t")
    nc.vector.memset(zero_t[:, :], 0.0)

    # ---- stage B: Q/K/V projections ----
    with tc.tile_pool(name="proj_psum", bufs=4, space="PSUM") as proj_psum:
        for fc in range(EC):
            for qw in range(NQ // 512):
                pq = proj_psum.tile([128, 512], f32, name="proj", tag="proj")
                for ec in range(EC):
                    nc.tensor.matmul(
                        pq[:, :],
                        w_bf["Wq"][:, E * ec + 128 * fc:E * ec + 128 * (fc + 1)],
                        ln1[ec][:, 512 * qw:512 * (qw + 1)],
                        start=(ec == 0), stop=(ec == EC - 1))
                nc.vector.tensor_copy(qt[fc][:, 512 * qw:512 * (qw + 1)], pq[:, :])
        for fc in range(EC):
            for kw in range(N // 512):
                pk = proj_psum.tile([128, 512], f32, name="proj", tag="proj")
                for ec in range(EC):
                    nc.tensor.matmul(
                        pk[:, :],
                        w_bf["Wk"][:, E * ec + 128 * fc:E * ec + 128 * (fc + 1)],
                        ln1[ec][:, 512 * kw:512 * (kw + 1)],
                        start=(ec == 0), stop=(ec == EC - 1))
                nc.scalar.copy(kt[fc][:, 512 * kw:512 * (kw + 1)], pk[:, :])
        for k in range(NKC):
            pv = proj_psum.tile([128, E], f32, name="projv", tag="projv")
            for ec in range(EC):
                nc.tensor.matmul(
                    pv[:, :],
                    ln1[ec][:, 128 * k:128 * (k + 1)],
                    w_bf["Wv"][:, E * ec:E * (ec + 1)],
                    start=(ec == 0), stop=(ec == EC - 1))
            nc.vector.tensor_copy(v_sb[k][:, :], pv[:, :])

    # ---- stage E: attention ----
    with tc.tile_pool(name="score_psum", bufs=1, space="PSUM") as sp, \
         tc.tile_pool(name="acc_psum", bufs=1, space="PSUM") as ap_, \
         tc.tile_pool(name="ef_sbuf", bufs=5) as efp, \
         tc.tile_pool(name="g_sbuf", bufs=6) as gp, \
         tc.tile_pool(name="att_sbuf", bufs=2) as asb:
        # Two independent 2-bank score tiles so writes for iteration i+1 only
        # wait on the matching half's exp of iteration i (true pipelining).
        S2 = [sp.tile([128, 2 * QC], f32, name=f"S{h}", tag=f"S{h}")
              for h in range(2)]
        for qc in range(2):
            q0 = QC * qc
            wv_ps = [ap_.tile([128, QC], f32, name=f"wv{s}", tag=f"wv{s}")
                     for s in range(2)]
            z_ps = [ap_.tile([128, QC], f32, name=f"z{s}", tag=f"z{s}")
                    for s in range(2)]
            pending = []
            for kc in range(NKC):
                gt = gp.tile([128, 2 * QC], bf16, name="lgg2", tag="lgg2")
                nc.sync.dma_start(gt[:, :],
                                  lgg2_d[128 * kc:128 * (kc + 1),
                                         2 * QC * qc:2 * QC * (qc + 1)])
                lg_t = gt[:, 0:QC]
                g2_t = gt[:, QC:2 * QC]
                for hg in range(2):
                    c = hg  # head group hg covers heads 4hg..4hg+3 = chunk hg
                    g2b = g2_t.rearrange("p (o q) -> p o q", o=1)\
                        .broadcast_to([128, 2, QC])
                    ef = []
                    # half A (heads 0,1 of group) in S2[0]; half B in S2[1]
                    for half in range(2):
                        S = S2[half]
                        for jj in range(2):
                            nc.tensor.matmul(
                                S[:, QC * jj:QC * (jj + 1)],
                                id_bf, lg_t,
                                start=True, stop=False)
                        for jj in range(2):
                            j = 2 * half + jj
                            hh = 32 * j
                            nc.tensor.matmul(
                                S[:, QC * jj:QC * (jj + 1)],
                                kt[c][hh:hh + 32, 128 * kc:128 * (kc + 1)],
                                qt[c][hh:hh + 32, q0:q0 + QC],
                                start=False, stop=True,
                                skip_group_check=True, tile_position=(hh, 0))
                    # flush previous iteration's Z/wv matmuls (deps all met)
                    for fn in pending:
                        fn()
                    pending = []
                    for half in range(2):
                        S = S2[half]
                        e = efp.tile([128, 2 * QC], bf16, name="e", tag="e")
                        f = efp.tile([128, 2 * QC], bf16, name="f", tag="f")
                        nc.scalar.activation(e[:, :], S[:, :], FT.Exp,
                                             bias=zero_t[:, :])
                        nc.vector.tensor_tensor(
                            f[:, :].rearrange("p (o q) -> p o q", o=2),
                            e[:, :].rearrange("p (o q) -> p o q", o=2),
                            g2b, ALU.mult)
                        ef.append((e, f))

                    def mk(kc=kc, hg=hg, ef=ef):
                        def emit():
                            for j in range(4):
                                e = ef[j // 2][0]
                                nc.tensor.matmul(
                                    z_ps[hg][32 * j:32 * j + 32, :],
                                    ones_bf[:, :],
                                    e[:, QC * (j % 2):QC * (j % 2 + 1)],
                                    start=(kc == 0), stop=(kc == NKC - 1),
                                    skip_group_check=True,
                                    tile_position=(0, 32 * j))
                            for j in range(4):
                                f = ef[j // 2][1]
                                h = 4 * hg + j
                                nc.tensor.matmul(
                                    wv_ps[hg][32 * j:32 * j + 32, :],
                                    v_sb[kc][:, 32 * h:32 * h + 32],
                                    f[:, QC * (j % 2):QC * (j % 2 + 1)],
                                    start=(kc == 0), stop=(kc == NKC - 1),
                                    skip_group_check=True,
                                    tile_position=(0, 32 * j))
                        return emit
                    pending.append(mk())
            for fn in pending:
                fn()
            # normalize + Wo projection + bias + residual -> h
            on = []
            for s in range(2):
                zr = asb.tile([128, QC], f32, name=f"zr{s}", tag=f"zr{s}")
                nc.vector.reciprocal_approx_fast(zr[:, :], z_ps[s][:, :])
                o = asb.tile([128, QC], bf16, name=f"on{s}", tag=f"on{s}")
                nc.vector.tensor_mul(o[:, :], wv_ps[s][:, :], zr[:, :])
                on.append(o)
            for fc in range(EC):
                po = ap_.tile([128, QC], f32, name="po", tag="z0")
                for ec in range(EC):
                    nc.tensor.matmul(
                        po[:, :],
                        w_bf["Wo"][:, E * ec + 128 * fc:E * ec + 128 * (fc + 1)],
                        on[ec][:, :],
                        start=(ec == 0), stop=(ec == EC - 1))
                nc.vector.scalar_tensor_tensor(
                    h_sb[fc][:, q0:q0 + QC], po[:, :],
                    vecs[:, 2 * V_BO + fc:2 * V_BO + fc + 1],
                    xtq[fc][:, q0:q0 + QC], ALU.add, ALU.add)

    # ---- stage F: LN2 + FFN + residual + store ----
    with tc.tile_pool(name="ln_psum2", bufs=2, space="PSUM") as ln_pp2, \
         tc.tile_pool(name="ffn_psum", bufs=2, space="PSUM") as fp, \
         tc.tile_pool(name="ffn_sbuf", bufs=2) as fs:
        # mean
        p_s = [ln_pp2.tile([128, 512], f32, name="lnps", tag="lnps")
               for _ in range(2)]
        for w in range(2):
            for c in range(EC):
                nc.tensor.matmul(p_s[w][:, :], ones[:, :],
                                 h_sb[c][:, 512 * w:512 * (w + 1)],
                                 start=(c == 0), stop=(c == EC - 1))
        mu = fs.tile([128, NQ], f32, name="mu", tag="mu")
        for w in range(2):
            nc.vector.tensor_scalar_mul(mu[:, 512 * w:512 * (w + 1)],
                                        p_s[w][:, :], 1.0 / E)
        # xm = h - mu ; var = mean(xm^2)
        xm = [fs.tile([128, NQ], f32, name=f"xm{c}", tag=f"xm{c}")
              for c in range(EC)]
        sqx = fs.tile([128, NQ], f32, name="sqx", tag="sqx")
        p_sq = [ln_pp2.tile([128, 512], f32, name="lnpsq", tag="lnpsq")
                for _ in range(2)]
        for c in range(EC):
            nc.vector.tensor_sub(xm[c][:, :], h_sb[c][:, :], mu[:, :])
        for c in range(EC):
            nc.vector.tensor_mul(sqx[:, :], xm[c][:, :], xm[c][:, :])
            for w in range(2):
                nc.tensor.matmul(p_sq[w][:, :], ones[:, :],
                                 sqx[:, 512 * w:512 * (w + 1)],
                                 start=(c == 0), stop=(c == EC - 1))
        var = fs.tile([128, NQ], f32, name="var", tag="var")
        for w in range(2):
            nc.vector.tensor_scalar_mul(var[:, 512 * w:512 * (w + 1)],
                                        p_sq[w][:, :], 1.0 / E)
        sd = fs.tile([128, NQ], f32, name="sd", tag="sd")
        nc.scalar.activation(sd[:, :], var[:, :], FT.Sqrt, bias=eps_t[:, :])
        rstd = fs.tile([128, NQ], f32, name="rstd", tag="rstd")
        nc.vector.reciprocal_approx_fast(rstd[:, :], sd[:, :])
        z1 = [fs.tile([128, NQ], bf16, name=f"z1{c}", tag=f"z1{c}")
              for c in range(EC)]
        ln2 = [fs.tile([128, NQ], bf16, name=f"ln2{c}", tag=f"ln2{c}")
               for c in range(EC)]
        for qw in range(2):
            sl = slice(512 * qw, 512 * (qw + 1))
            for c in range(EC):
                xm2 = fs.tile([128, 512], f32, name="xm2", tag="xm2")
                nc.vector.tensor_mul(xm2[:, :], xm[c][:, sl], rstd[:, sl])
                nc.vector.tensor_scalar(
                    ln2[c][:, sl], xm2[:, :],
                    vecs[:, 2 * V_G2 + c:2 * V_G2 + c + 1],
                    vecs[:, 2 * V_BETA2 + c:2 * V_BETA2 + c + 1],
                    ALU.mult, ALU.add)
            for fc in range(EC):
                p1 = fp.tile([128, 512], f32, name="ffn", tag="ffn")
                for ec in range(EC):
                    nc.tensor.matmul(
                        p1[:, :],
                        w_bf["W1"][:, E * ec + 128 * fc:E * ec + 128 * (fc + 1)],
                        ln2[ec][:, sl],
                        start=(ec == 0), stop=(ec == EC - 1))
                nc.vector.tensor_scalar(z1[fc][:, sl], p1[:, :],
                                        vecs[:, 2 * V_B1 + fc:2 * V_B1 + fc + 1],
                                        0.0, ALU.add, ALU.max)
            for fc in range(EC):
                p2 = fp.tile([128, 512], f32, name="ffn", tag="ffn")
                for ec in range(EC):
                    nc.tensor.matmul(
                        p2[:, :],
                        w_bf["W2"][:, E * ec + 128 * fc:E * ec + 128 * (fc + 1)],
                        z1[ec][:, sl],
                        start=(ec == 0), stop=(ec == EC - 1))
                of = fs.tile([128, 512], f32, name="of", tag="of")
                nc.vector.scalar_tensor_tensor(
                    of[:, :], p2[:, :],
                    vecs[:, 2 * V_B2 + fc:2 * V_B2 + fc + 1],
                    h_sb[fc][:, sl], ALU.add, ALU.add)
                nc.sync.dma_start(
                    outT_d[128 * fc:128 * (fc + 1), sl],
                    of[:, :])

    for p in reversed(persist_pools):
        p.__exit__(None, None, None)


def build_nc():
    nc = bacc.Bacc(
        "TRN2",
        target_bir_lowering=False,
        debug=False,
        enable_asserts=False,
        num_devices=8,
    )
    ln1b_d = nc.dram_tensor("ln1b", [128, 2 * N], bf16, kind="ExternalInput").ap()
    xqb_d = nc.dram_tensor("xqb", [128, 2 * NQ], f32, kind="ExternalInput").ap()
    lgg2_d = nc.dram_tensor("lgg2", [N, 2 * NQ], bf16, kind="ExternalInput").ap()
    wblob_d = nc.dram_tensor("wblob", [128, 3200], bf16, kind="ExternalInput").ap()
    vecs_d = nc.dram_tensor("vecs", [128, 10], f32, kind="ExternalInput").ap()
    outT_d = nc.dram_tensor("outT", [E, NQ], f32, kind="ExternalOutput").ap()

    with tile.TileContext(nc) as tc:
        build_body(nc, tc, ln1b_d, xqb_d, lgg2_d, wblob_d, vecs_d, outT_d)
    nc.compile()
    return nc


def host_shard(inputs):
    """Build the 8 per-core input maps (see module docstring)."""
    x = np.asarray(inputs["x"], np.float32)
    infl = np.asarray(inputs["influence_matrix"], np.float32)
    iw1 = float(np.asarray(inputs["iw1"]))
    ib1 = float(np.asarray(inputs["ib1"]))
    iw2 = float(np.asarray(inputs["iw2"]))
    ib2 = float(np.asarray(inputs["ib2"]))
    g1 = np.asarray(inputs["g1"], np.float32).reshape(E)
    beta1 = np.asarray(inputs["beta1"], np.float32).reshape(E)

    vec_list = ["g2", "beta2", "bo", "b1", "b2"]
    vecs_np = np.empty((128, 10), np.float32)
    for vi, nm in enumerate(vec_list):
        v = np.asarray(inputs[nm], np.float32).reshape(E)
        vecs_np[:, 2 * vi] = v[:128]
        vecs_np[:, 2 * vi + 1] = v[128:]

    # weight blob: 6 matrices as [128, 512] (chunk-major cols) + identity
    wblob = np.zeros((128, 3200), np.float32)
    for i, n in enumerate(W_NAMES):
        w = np.asarray(inputs[n], np.float32)
        if n == "Wq":
            w = w / math.sqrt(D)
        for c in range(EC):
            wblob[:, 512 * i + E * c:512 * i + E * (c + 1)] = w[128 * c:128 * (c + 1), :]
    wblob[:, 3072:3200] = np.eye(128, dtype=np.float32)
    wblob = wblob.astype(ml_dtypes.bfloat16)

    # host LN1 (input prep: pure function of inputs x, g1, beta1)
    mu = x.mean(axis=-1, keepdims=True)
    var = x.var(axis=-1, keepdims=True)
    ln1 = (x - mu) / np.sqrt(var + 1e-5) * g1 + beta1  # [B, N, E] f32

    in_maps = []
    for core in range(8):
        b, qh = core // 2, core % 2
        qoff = qh * NQ
        ln1b = np.roll(ln1[b], -qoff, axis=0)          # [N, E]
        ln1T = ln1b.T                                  # [E, N]
        ln1_pack = np.empty((128, 2 * N), np.float32)
        for c in range(EC):
            ln1_pack[:, N * c:N * (c + 1)] = ln1T[128 * c:128 * (c + 1), :]
        xqT = x[b][qoff:qoff + NQ].T                   # [E, NQ]
        xq_pack = np.empty((128, 2 * NQ), np.float32)
        for c in range(EC):
            xq_pack[:, NQ * c:NQ * (c + 1)] = xqT[128 * c:128 * (c + 1), :]
        inf_slice = np.roll(infl[b][qoff:qoff + NQ, :], -qoff, axis=1)
        infT = inf_slice.T                              # [N(k), NQ(q)]
        lgg2 = np.empty((N, 2 * NQ), np.float32)
        for q in range(2):
            lgg2[:, 1024 * q:1024 * q + 512] = iw1 * infT[:, 512 * q:512 * (q + 1)] + ib1
            lgg2[:, 1024 * q + 512:1024 * (q + 1)] = iw2 * infT[:, 512 * q:512 * (q + 1)] + ib2
        m = {"ln1b": ln1_pack.astype(ml_dtypes.bfloat16),
             "xqb": np.ascontiguousarray(xq_pack),
             "lgg2": lgg2.astype(ml_dtypes.bfloat16),
             "wblob": wblob, "vecs": vecs_np}
        in_maps.append(m)
    return in_maps


_NC_CACHE = []


def kernel(**inputs):
    if not _NC_CACHE:
        _NC_CACHE.append(build_nc())
    nc = _NC_CACHE[0]
    in_maps = host_shard(inputs)
    res = run_bass_kernel_spmd(nc, in_maps, core_ids=list(range(8)))
    out = np.empty((B, N, E), np.float32)
    for core in range(8):
        b, qh = core // 2, core % 2
        out[b, qh * NQ:(qh + 1) * NQ, :] = np.asarray(
            res.results[core]["outT"], np.float32).T
    return out


# revision 21
# speedup vs baseline: 1.8122x; 1.0357x over previous
"""Graphormer layer on 8 TRN2 NeuronCores — v4 (ACT-bound, packed DMA).

Sharding: core c handles batch b = c//2 and query-row half qh = c%2 (1024 q
rows). Transposed (feature-on-partition) layout throughout; host pre-rolls
the node axis per core so each core's own q rows sit first, and ships:
  - ln1b  : LayerNorm1(x) pre-computed, transposed, bf16 (packed chunks)
  - xqb   : x^T own-query columns (residual), f32 (packed chunks)
  - lgg2  : [(iw1*u+ib1) | (iw2*u+ib2)]^T per-core slice, bf16, fused
  - wblob : all six weight matrices (Wq pre-scaled 1/sqrt(D)) + identity

Device per core:
  B:  QT/KT/V projections from ln1 (bf16 matmuls; kt casts on ScalarE)
  E:  per (qc, kc, hg): two 2-bank score tiles S2[half] get LG preload
      (identity matmul) + 2 row-packed QK matmuls each; exp on [128,1024]
      (ACT = bottleneck ~147us); f = e*G2 (DVE 2x); Z/wv via 4-way
      col-packed matmuls emitted with one-iteration lag (PE FIFO never
      stalls on ACT/DVE).
  F:  attn = (wv/Z) @ Wo + bo + x ; LN2 + FFN + residual, store.
"""

import math

import numpy as np
import ml_dtypes

import concourse.bass as bass
import concourse.bacc as bacc
import concourse.mybir as mybir
import concourse.tile as tile
from concourse.bass_utils import run_bass_kernel_spmd

B, N, E, H, D = 4, 2048, 256, 8, 32
NQ = N // 2          # q rows per core
QC = 512             # q window
NKC = N // 128       # 16 k-chunks
EC = E // 128        # 2 feature chunks

f32 = mybir.dt.float32
bf16 = mybir.dt.bfloat16
FT = mybir.ActivationFunctionType
ALU = mybir.AluOpType

V_G2, V_BETA2, V_BO, V_B1, V_B2 = range(5)
W_NAMES = ("Wq", "Wk", "Wv", "Wo", "W1", "W2")


def build_body(nc, tc, ln1b_d, xqb_d, lgg2_d, wblob_d, vecs_d, outT_d):
    persist_pools = []

    def ppool(name, space="SBUF"):
        p = tc.tile_pool(name=name, bufs=1, space=space)
        persist_pools.append(p)
        return p.__enter__()

    persist = ppool("persist")

    # ---- persistent SBUF ----
    qt = [persist.tile([128, NQ], bf16, name=f"qt{c}", tag=f"qt{c}") for c in range(EC)]
    kt = [persist.tile([128, N], bf16, name=f"kt{c}", tag=f"kt{c}") for c in range(EC)]
    v_sb = [persist.tile([128, E], bf16, name=f"v{k}", tag=f"v{k}") for k in range(NKC)]
    wblob = persist.tile([128, 3200], bf16, name="wblob", tag="wblob")
    lnb = persist.tile([128, 2 * N], bf16, name="lnb", tag="lnb")
    xqb = persist.tile([128, 2 * NQ], f32, name="xqb", tag="xqb")
    vecs = persist.tile([128, 10], f32, name="vecs", tag="vecs")
    ones = persist.tile([128, 128], f32, name="ones", tag="ones")
    ones_bf = persist.tile([128, 32], bf16, name="ones_bf", tag="ones_bf")
    h_sb = [persist.tile([128, NQ], f32, name=f"h{c}", tag=f"h{c}") for c in range(EC)]

    w_bf = {n: wblob[:, 512 * i:512 * (i + 1)] for i, n in enumerate(W_NAMES)}
    id_bf = wblob[:, 3072:3200]
    ln1 = [lnb[:, N * c:N * (c + 1)] for c in range(EC)]
    xtq = [xqb[:, NQ * c:NQ * (c + 1)] for c in range(EC)]

    # ---- packed loads ----
    nc.sync.dma_start(wblob[:, :], wblob_d[:, :])
    nc.sync.dma_start(lnb[:, :], ln1b_d[:, :])
    nc.sync.dma_start(xqb[:, :], xqb_d[:, :])
    nc.sync.dma_start(vecs[:, :], vecs_d[:, :])
    nc.vector.memset(ones[:, :], 1.0)
    nc.vector.memset(ones_bf[:, :], 1.0)
    eps_t = persist.tile([128, 1], f32, name="eps_t", tag="eps_t")
    nc.vector.memset(eps_t[:, :], 1e-5)
    zero_t = persist.tile([128, 1], f32, name="zero_t", tag="zero_t")
    nc.vector.memset(zero_t[:, :], 0.0)

    # ---- stage B: Q/K/V projections ----
    with tc.tile_pool(name="proj_psum", bufs=4, space="PSUM") as proj_psum:
        # PE warmup during the input DMA wait: ~6us of dummy matmuls gets the
        # HAM clock gate to 8/8 before the real work arrives.
        wsrc = persist.tile([128, 512], bf16, name="wsrc", tag="wsrc")
        nc.vector.memset(wsrc[:, :], 1.0)
        for r in range(14):
            wm = proj_psum.tile([128, 512], f32, name="warm", tag="proj")
            nc.tensor.matmul(wm[:, :], wsrc[:, 0:128], wsrc[:, :],
                             start=True, stop=True)
        for fc in range(EC):
            for qw in range(NQ // 512):
                pq = proj_psum.tile([128, 512], f32, name="proj", tag="proj")
                for ec in range(EC):
                    nc.tensor.matmul(
                        pq[:, :],
                        w_bf["Wq"][:, E * ec + 128 * fc:E * ec + 128 * (fc + 1)],
                        ln1[ec][:, 512 * qw:512 * (qw + 1)],
                        start=(ec == 0), stop=(ec == EC - 1))
                nc.vector.tensor_copy(qt[fc][:, 512 * qw:512 * (qw + 1)], pq[:, :])
        for fc in range(EC):
            for kw in range(N // 512):
                pk = proj_psum.tile([128, 512], f32, name="proj", tag="proj")
                for ec in range(EC):
                    nc.tensor.matmul(
                        pk[:, :],
                        w_bf["Wk"][:, E * ec + 128 * fc:E * ec + 128 * (fc + 1)],
                        ln1[ec][:, 512 * kw:512 * (kw + 1)],
                        start=(ec == 0), stop=(ec == EC - 1))
                nc.scalar.copy(kt[fc][:, 512 * kw:512 * (kw + 1)], pk[:, :])
        for k in range(NKC):
            pv = proj_psum.tile([128, E], f32, name="projv", tag="projv")
            for ec in range(EC):
                nc.tensor.matmul(
                    pv[:, :],
                    ln1[ec][:, 128 * k:128 * (k + 1)],
                    w_bf["Wv"][:, E * ec:E * (ec + 1)],
                    start=(ec == 0), stop=(ec == EC - 1))
            nc.vector.tensor_copy(v_sb[k][:, :], pv[:, :])

    # ---- stage E: attention ----
    with tc.tile_pool(name="score_psum", bufs=1, space="PSUM") as sp, \
         tc.tile_pool(name="acc_psum", bufs=1, space="PSUM") as ap_, \
         tc.tile_pool(name="ef_sbuf", bufs=5) as efp, \
         tc.tile_pool(name="g_sbuf", bufs=6) as gp, \
         tc.tile_pool(name="att_sbuf", bufs=2) as asb:
        # Two independent 2-bank score tiles so writes for iteration i+1 only
        # wait on the matching half's exp of iteration i (true pipelining).
        S2 = [sp.tile([128, 2 * QC], f32, name=f"S{h}", tag=f"S{h}")
              for h in range(2)]
        drain_pending = []
        for qc in range(2):
            q0 = QC * qc
            wv_ps = [ap_.tile([128, QC], f32, name=f"wv{s}", tag=f"wv{s}")
                     for s in range(2)]
            z_ps = [ap_.tile([128, QC], f32, name=f"z{s}", tag=f"z{s}")
                    for s in range(2)]
            pending = []
            for kc in range(NKC):
                gt = gp.tile([128, 2 * QC], bf16, name="lgg2", tag="lgg2")
                nc.sync.dma_start(gt[:, :],
                                  lgg2_d[128 * kc:128 * (kc + 1),
                                         2 * QC * qc:2 * QC * (qc + 1)])
                lg_t = gt[:, 0:QC]
                g2_t = gt[:, QC:2 * QC]
                for hg in range(2):
                    c = hg  # head group hg covers heads 4hg..4hg+3 = chunk hg
                    g2b = g2_t.rearrange("p (o q) -> p o q", o=1)\
                        .broadcast_to([128, 2, QC])
                    ef = []
                    # half A (heads 0,1 of group) in S2[0]; half B in S2[1]
                    for half in range(2):
                        S = S2[half]
                        for jj in range(2):
                            nc.tensor.matmul(
                                S[:, QC * jj:QC * (jj + 1)],
                                id_bf, lg_t,
                                start=True, stop=False)
                        for jj in range(2):
                            j = 2 * half + jj
                            hh = 32 * j
                            nc.tensor.matmul(
                                S[:, QC * jj:QC * (jj + 1)],
                                kt[c][hh:hh + 32, 128 * kc:128 * (kc + 1)],
                                qt[c][hh:hh + 32, q0:q0 + QC],
                                start=False, stop=True,
                                skip_group_check=True, tile_position=(hh, 0))
                    # flush previous iteration's Z/wv matmuls (deps all met)
                    for fn in pending:
                        fn()
                    pending = []
                    # previous q-window's drain goes here so its Wo matmuls
                    # don't block this window's first scores in the PE FIFO
                    for fn in drain_pending:
                        fn()
                    drain_pending = []
                    for half in range(2):
                        S = S2[half]
                        e = efp.tile([128, 2 * QC], bf16, name="e", tag="e")
                        f = efp.tile([128, 2 * QC], bf16, name="f", tag="f")
                        nc.scalar.activation(e[:, :], S[:, :], FT.Exp,
                                             bias=zero_t[:, :])
                        nc.vector.tensor_tensor(
                            f[:, :].rearrange("p (o q) -> p o q", o=2),
                            e[:, :].rearrange("p (o q) -> p o q", o=2),
                            g2b, ALU.mult)
                        ef.append((e, f))

                    def mk(kc=kc, hg=hg, ef=ef):
                        def emit():
                            for j in range(4):
                                e = ef[j // 2][0]
                                nc.tensor.matmul(
                                    z_ps[hg][32 * j:32 * j + 32, :],
                                    ones_bf[:, :],
                                    e[:, QC * (j % 2):QC * (j % 2 + 1)],
                                    start=(kc == 0), stop=(kc == NKC - 1),
                                    skip_group_check=True,
                                    tile_position=(0, 32 * j))
                            for j in range(4):
                                f = ef[j // 2][1]
                                h = 4 * hg + j
                                nc.tensor.matmul(
                                    wv_ps[hg][32 * j:32 * j + 32, :],
                                    v_sb[kc][:, 32 * h:32 * h + 32],
                                    f[:, QC * (j % 2):QC * (j % 2 + 1)],
                                    start=(kc == 0), stop=(kc == NKC - 1),
                                    skip_group_check=True,
                                    tile_position=(0, 32 * j))
                        return emit
                    pending.append(mk())
            for fn in pending:
                fn()

            # normalize + Wo projection + bias + residual -> h (deferred:
            # emitted after the next q-window's first scores, or at loop end)
            def mk_drain(qc=qc, q0=q0, wv_ps=wv_ps, z_ps=z_ps):
                def emit():
                    on = []
                    for s in range(2):
                        zr = asb.tile([128, QC], f32, name=f"zr{s}", tag=f"zr{s}")
                        nc.vector.reciprocal_approx_fast(zr[:, :], z_ps[s][:, :])
                        o = asb.tile([128, QC], bf16, name=f"on{s}", tag=f"on{s}")
                        nc.vector.tensor_mul(o[:, :], wv_ps[s][:, :], zr[:, :])
                        on.append(o)
                    for fc in range(EC):
                        po = sp.tile([128, QC], f32, name="po", tag=f"S{fc}")
                        for ec in range(EC):
                            nc.tensor.matmul(
                                po[:, :],
                                w_bf["Wo"][:, E * ec + 128 * fc:E * ec + 128 * (fc + 1)],
                                on[ec][:, :],
                                start=(ec == 0), stop=(ec == EC - 1))
                        nc.vector.scalar_tensor_tensor(
                            h_sb[fc][:, q0:q0 + QC], po[:, :],
                            vecs[:, 2 * V_BO + fc:2 * V_BO + fc + 1],
                            xtq[fc][:, q0:q0 + QC], ALU.add, ALU.add)
                return emit
            drain_pending.append(mk_drain())
        for fn in drain_pending:
            fn()

    # ---- stage F: LN2 + FFN + residual + store ----
    with tc.tile_pool(name="ln_psum2", bufs=2, space="PSUM") as ln_pp2, \
         tc.tile_pool(name="ffn_psum", bufs=2, space="PSUM") as fp, \
         tc.tile_pool(name="ffn_sbuf", bufs=2) as fs:
        # bf16 copies of h for cheap stats (PE accumulates fp32 in PSUM)
        hb = [fs.tile([128, NQ], bf16, name=f"hb{c}", tag=f"hb{c}")
              for c in range(EC)]
        for c in range(EC):
            nc.vector.tensor_copy(hb[c][:, :], h_sb[c][:, :])
        p_s = [ln_pp2.tile([128, 512], f32, name="lnps", tag="lnps")
               for _ in range(2)]
        for w in range(2):
            for c in range(EC):
                nc.tensor.matmul(p_s[w][:, :], wsrc[:, 0:128],
                                 hb[c][:, 512 * w:512 * (w + 1)],
                                 start=(c == 0), stop=(c == EC - 1))
        mu = fs.tile([128, NQ], bf16, name="mu", tag="mu")
        for w in range(2):
            nc.vector.tensor_scalar_mul(mu[:, 512 * w:512 * (w + 1)],
                                        p_s[w][:, :], 1.0 / E)
        # xm = h - mu ; var = mean(xm^2)
        xm = [fs.tile([128, NQ], bf16, name=f"xm{c}", tag=f"xm{c}")
              for c in range(EC)]
        sqx = fs.tile([128, NQ], bf16, name="sqx", tag="sqx")
        p_sq = [ln_pp2.tile([128, 512], f32, name="lnpsq", tag="lnpsq")
                for _ in range(2)]
        for c in range(EC):
            nc.vector.tensor_sub(xm[c][:, :], hb[c][:, :], mu[:, :])
        for c in range(EC):
            nc.vector.tensor_mul(sqx[:, :], xm[c][:, :], xm[c][:, :])
            for w in range(2):
                nc.tensor.matmul(p_sq[w][:, :], wsrc[:, 0:128],
                                 sqx[:, 512 * w:512 * (w + 1)],
                                 start=(c == 0), stop=(c == EC - 1))
        var = fs.tile([128, NQ], f32, name="var", tag="var")
        for w in range(2):
            nc.vector.tensor_scalar_mul(var[:, 512 * w:512 * (w + 1)],
                                        p_sq[w][:, :], 1.0 / E)
        sd = fs.tile([128, NQ], f32, name="sd", tag="sd")
        nc.scalar.activation(sd[:, :], var[:, :], FT.Sqrt, bias=eps_t[:, :])
        rstdf = fs.tile([128, NQ], f32, name="rstdf", tag="rstdf")
        nc.vector.reciprocal_approx_fast(rstdf[:, :], sd[:, :])
        rstd = fs.tile([128, NQ], bf16, name="rstd", tag="rstd")
        nc.vector.tensor_copy(rstd[:, :], rstdf[:, :])
        z1 = [fs.tile([128, NQ], bf16, name=f"z1{c}", tag=f"z1{c}")
              for c in range(EC)]
        ln2 = [fs.tile([128, NQ], bf16, name=f"ln2{c}", tag=f"ln2{c}")
               for c in range(EC)]
        for qw in range(2):
            sl = slice(512 * qw, 512 * (qw + 1))
            for c in range(EC):
                xm2 = fs.tile([128, 512], bf16, name="xm2", tag="xm2")
                nc.vector.tensor_mul(xm2[:, :], xm[c][:, sl], rstd[:, sl])
                nc.vector.tensor_scalar(
                    ln2[c][:, sl], xm2[:, :],
                    vecs[:, 2 * V_G2 + c:2 * V_G2 + c + 1],
                    vecs[:, 2 * V_BETA2 + c:2 * V_BETA2 + c + 1],
                    ALU.mult, ALU.add)
            for fc in range(EC):
                p1 = fp.tile([128, 512], f32, name="ffn", tag="ffn")
                for ec in range(EC):
                    nc.tensor.matmul(
                        p1[:, :],
                        w_bf["W1"][:, E * ec + 128 * fc:E * ec + 128 * (fc + 1)],
                        ln2[ec][:, sl],
                        start=(ec == 0), stop=(ec == EC - 1))
                nc.vector.tensor_scalar(z1[fc][:, sl], p1[:, :],
                                        vecs[:, 2 * V_B1 + fc:2 * V_B1 + fc + 1],
                                        0.0, ALU.add, ALU.max)
            for fc in range(EC):
                p2 = fp.tile([128, 512], f32, name="ffn", tag="ffn")
                for ec in range(EC):
                    nc.tensor.matmul(
                        p2[:, :],
                        w_bf["W2"][:, E * ec + 128 * fc:E * ec + 128 * (fc + 1)],
                        z1[ec][:, sl],
                        start=(ec == 0), stop=(ec == EC - 1))
                of = fs.tile([128, 512], f32, name="of", tag="of")
                nc.vector.scalar_tensor_tensor(
                    of[:, :], p2[:, :],
                    vecs[:, 2 * V_B2 + fc:2 * V_B2 + fc + 1],
                    h_sb[fc][:, sl], ALU.add, ALU.add)
                nc.sync.dma_start(
                    outT_d[128 * fc:128 * (fc + 1), sl],
                    of[:, :])

    for p in reversed(persist_pools):
        p.__exit__(None, None, None)


def build_nc():
    nc = bacc.Bacc(
        "TRN2",
        target_bir_lowering=False,
        debug=False,
        enable_asserts=False,
        num_devices=8,
    )
    ln1b_d = nc.dram_tensor("ln1b", [128, 2 * N], bf16, kind="ExternalInput").ap()
    xqb_d = nc.dram_tensor("xqb", [128, 2 * NQ], f32, kind="ExternalInput").ap()
    lgg2_d = nc.dram_tensor("lgg2", [N, 2 * NQ], bf16, kind="ExternalInput").ap()
    wblob_d = nc.dram_tensor("wblob", [128, 3200], bf16, kind="ExternalInput").ap()
    vecs_d = nc.dram_tensor("vecs", [128, 10], f32, kind="ExternalInput").ap()
    outT_d = nc.dram_tensor("outT", [E, NQ], f32, kind="ExternalOutput").ap()

    with tile.TileContext(nc) as tc:
        build_body(nc, tc, ln1b_d, xqb_d, lgg2_d, wblob_d, vecs_d, outT_d)
    nc.compile()
    return nc


def host_shard(inputs):
    """Build the 8 per-core input maps (see module docstring)."""
    x = np.asarray(inputs["x"], np.float32)
    infl = np.asarray(inputs["influence_matrix"], np.float32)
    iw1 = float(np.asarray(inputs["iw1"]))
    ib1 = float(np.asarray(inputs["ib1"]))
    iw2 = float(np.asarray(inputs["iw2"]))
    ib2 = float(np.asarray(inputs["ib2"]))
    g1 = np.asarray(inputs["g1"], np.float32).reshape(E)
    beta1 = np.asarray(inputs["beta1"], np.float32).reshape(E)

    vec_list = ["g2", "beta2", "bo", "b1", "b2"]
    vecs_np = np.empty((128, 10), np.float32)
    for vi, nm in enumerate(vec_list):
        v = np.asarray(inputs[nm], np.float32).reshape(E)
        vecs_np[:, 2 * vi] = v[:128]
        vecs_np[:, 2 * vi + 1] = v[128:]

    # weight blob: 6 matrices as [128, 512] (chunk-major cols) + identity
    wblob = np.zeros((128, 3200), np.float32)
    for i, n in enumerate(W_NAMES):
        w = np.asarray(inputs[n], np.float32)
        if n == "Wq":
            w = w / math.sqrt(D)
        for c in range(EC):
            wblob[:, 512 * i + E * c:512 * i + E * (c + 1)] = w[128 * c:128 * (c + 1), :]
    wblob[:, 3072:3200] = np.eye(128, dtype=np.float32)
    wblob = wblob.astype(ml_dtypes.bfloat16)

    # host LN1 (input prep: pure function of inputs x, g1, beta1)
    mu = x.mean(axis=-1, keepdims=True)
    var = x.var(axis=-1, keepdims=True)
    ln1 = (x - mu) / np.sqrt(var + 1e-5) * g1 + beta1  # [B, N, E] f32

    in_maps = []
    for core in range(8):
        b, qh = core // 2, core % 2
        qoff = qh * NQ
        ln1b = np.roll(ln1[b], -qoff, axis=0)          # [N, E]
        ln1T = ln1b.T                                  # [E, N]
        ln1_pack = np.empty((128, 2 * N), np.float32)
        for c in range(EC):
            ln1_pack[:, N * c:N * (c + 1)] = ln1T[128 * c:128 * (c + 1), :]
        xqT = x[b][qoff:qoff + NQ].T                   # [E, NQ]
        xq_pack = np.empty((128, 2 * NQ), np.float32)
        for c in range(EC):
            xq_pack[:, NQ * c:NQ * (c + 1)] = xqT[128 * c:128 * (c + 1), :]
        inf_slice = np.roll(infl[b][qoff:qoff + NQ, :], -qoff, axis=1)
        infT = inf_slice.T                              # [N(k), NQ(q)]
        lgg2 = np.empty((N, 2 * NQ), np.float32)
        for q in range(2):
            lgg2[:, 1024 * q:1024 * q + 512] = iw1 * infT[:, 512 * q:512 * (q + 1)] + ib1
            lgg2[:, 1024 * q + 512:1024 * (q + 1)] = iw2 * infT[:, 512 * q:512 * (q + 1)] + ib2
        m = {"ln1b": ln1_pack.astype(ml_dtypes.bfloat16),
             "xqb": np.ascontiguousarray(xq_pack),
             "lgg2": lgg2.astype(ml_dtypes.bfloat16),
             "wblob": wblob, "vecs": vecs_np}
        in_maps.append(m)
    return in_maps


_NC_CACHE = []


def kernel(**inputs):
    if not _NC_CACHE:
        _NC_CACHE.append(build_nc())
    nc = _NC_CACHE[0]
    in_maps = host_shard(inputs)
    res = run_bass_kernel_spmd(nc, in_maps, core_ids=list(range(8)))
    out = np.empty((B, N, E), np.float32)
    for core in range(8):
        b, qh = core // 2, core % 2
        out[b, qh * NQ:(qh + 1) * NQ, :] = np.asarray(
            res.results[core]["outT"], np.float32).T
    return out
